# revision 1
# baseline (speedup 1.0000x reference)
"""Fused pre-LN transformer block (causal MHA + FFN) on 8 TRN2 NeuronCores.

Sharding: core c handles batch b = c//2 and head-half hh = c%2 (8 of 16 heads).
Attention runs fully local per (batch, head-half); the attention projection
produces a partial sum that is ReduceScattered (over token dim) within each
core pair, so FFN runs token-sharded (1024 tokens/core). Output per core is
its token slice, stored E-major [E, TH]; the host transposes when gathering.

v2: restructured to minimize instruction count (the backend cost model is
~flat per instruction): big-tile vector ops, weight-stationary matmul
ordering (Ldweights dedupe), paired 2-bank PSUM tiles (half the exp/relu
count), constant causal masks, e-major post-attention pipeline (no PE
transposes), whole-tensor DMAs.
"""

import numpy as np
import ml_dtypes

import concourse.bass as bass
import concourse.mybir as mybir
import concourse.tile as tile
from concourse import bacc
from concourse.bass import ts, ds
from concourse.bass_utils import run_bass_kernel_spmd

BF16 = mybir.dt.bfloat16
F32 = mybir.dt.float32
NPBF16 = ml_dtypes.bfloat16

B, T, E = 4, 2048, 1024
H, HS = 16, 64
FF = 4 * E
EPS = 1e-5
NCORES = 8
HPC = 8            # heads per core
HD = HPC * HS      # 512 head dims per core
TH = T // 2        # 1024 tokens per core for FFN
KT_N = T // 128    # 16 k-tiles
ET = E // 128      # 8 e-tiles
ADD = mybir.AluOpType.add
SUB = mybir.AluOpType.subtract
MUL = mybir.AluOpType.mult
EXP = mybir.ActivationFunctionType.Exp
RELU = mybir.ActivationFunctionType.Relu
SQRT = mybir.ActivationFunctionType.Sqrt
COPY = mybir.ActivationFunctionType.Copy


def _bc3(nc, pool, row, n_mid, n_free, tag):
    """[1, n_free] row -> materialized [128, n_free] bcast tile, viewed as a
    stride-0-middle-dim [128, n_mid, n_free] AP."""
    bc = pool.tile([128, n_free], F32, tag=tag, bufs=1, name=tag)
    nc.gpsimd.partition_broadcast(bc, row)
    return bc.unsqueeze(1).broadcast_to([128, n_mid, n_free])


def build_program(single=False, body=True, loop_n=1, no_cc=False,
                  bench_in=None, skip_attn=False, skip_ffn=False,
                  skip_qkv=False):
    nc = bacc.Bacc("TRN2", target_bir_lowering=False, debug=False,
                   num_devices=1 if single else NCORES)

    # ---- I/O ----
    if bench_in is None:
        xT = nc.dram_tensor("xT", [E, T], BF16, kind="ExternalInput").ap()
        xrpT = nc.dram_tensor("xrpT", [E, TH], F32, kind="ExternalInput").ap()
        qw = nc.dram_tensor("qw", [E, HD], BF16, kind="ExternalInput").ap()
        kw = nc.dram_tensor("kw", [E, HD], BF16, kind="ExternalInput").ap()
        vw = nc.dram_tensor("vw", [E, HD], BF16, kind="ExternalInput").ap()
        apw = nc.dram_tensor("apw", [HD, E], BF16, kind="ExternalInput").ap()
        fw1 = nc.dram_tensor("fw1", [E, FF], BF16, kind="ExternalInput").ap()
        fb1 = nc.dram_tensor("fb1", [128, FF // 128], F32,
                             kind="ExternalInput").ap()
        fw2 = nc.dram_tensor("fw2", [FF, E], BF16, kind="ExternalInput").ap()
        fb2 = nc.dram_tensor("fb2", [128, ET], F32, kind="ExternalInput").ap()
        out = nc.dram_tensor("out", [E, TH], F32, kind="ExternalOutput").ap()
        tok = None
    else:
        _c = {k: nc.inline_tensor(np.ascontiguousarray(v), k).ap()
              for k, v in bench_in.items()}
        xT, xrpT, qw, kw, vw, apw, fw1, fb1, fw2, fb2 = (
            _c["xT"], _c["xrpT"], _c["qw"], _c["kw"], _c["vw"], _c["apw"],
            _c["fw1"], _c["fb1"], _c["fw2"], _c["fb2"])
        out = nc.dram_tensor("out", [E, TH], F32).ap()
        tok = nc.dram_tensor("tok", [1, 128], F32, kind="ExternalOutput").ap()

    # internal DRAM for the pair-wise reduce-scatter.
    # part layout [2(half), E, TH(tok-local)]: RS splits dim0.
    part = nc.dram_tensor("part", [2, E, TH], F32).ap()
    rs = nc.dram_tensor("rs", [E, TH], F32).ap()
    groups = [[0, 1], [2, 3], [4, 5], [6, 7]]

    if not body:
        with tile.TileContext(nc) as tc:
            with tc.tile_pool(name="trivial", bufs=1) as pool:
                t = pool.tile([128, 8, TH], F32)
                nc.sync.dma_start(
                    out=t, in_=xrpT.rearrange("(i p) t -> p i t", p=128))
                nc.vector.tensor_scalar_mul(t, t, 1.0)
                nc.sync.dma_start(
                    out=out.rearrange("(i p) t -> p i t", p=128), in_=t)
                if tok is not None:
                    nc.sync.dma_start(out=tok, in_=out[0:1, 0:128])
        nc.compile()
        return nc

    def _emit(tc):
        with tc.tile_pool(name="const", bufs=1) as constp:
            ones_bf = constp.tile([128, 1], BF16)
            nc.gpsimd.memset(ones_bf, 1.0)
            eps_sb = constp.tile([1, 1], F32)
            nc.gpsimd.memset(eps_sb, EPS)
            # causal masks for the 4 diagonal k-offsets:
            # masks[kk, m, qq] = 1 if qq >= 128*m + kk else 0
            masks = constp.tile([128, 4, 512], BF16)
            nc.gpsimd.memset(masks, 1.0)
            nc.gpsimd.affine_select(
                out=masks, in_=masks, compare_op=mybir.AluOpType.is_ge,
                fill=0.0, base=0, pattern=[[-128, 4], [1, 512]],
                channel_multiplier=-1)
            fb1_sb = constp.tile([128, FF // 128], F32)
            nc.sync.dma_start(out=fb1_sb, in_=fb1)
            fb2_sb = constp.tile([128, ET], F32)
            nc.sync.dma_start(out=fb2_sb, in_=fb2)

            # x' (post-attention residual input), e-major, lives to the end
            xp = constp.tile([128, ET, TH], F32, name="xp")

            # ======== phase A: attention ========
            with tc.tile_pool(name="persA", bufs=1) as pA:
                AO = pA.tile([128, 4, T], BF16, name="AO")
                with tc.tile_pool(name="persA1", bufs=1) as pA1:
                    QT = pA1.tile([128, 4, T], BF16, name="QT")
                    KT = pA1.tile([128, 4, T], BF16, name="KT")
                    Vp = pA1.tile([128, KT_N, HPC, 65], BF16, name="Vp")
                    nc.vector.memset(Vp[:, :, :, 64:65], 1.0)

                    # ---- A1: LN1 + QKV over full T, e-major ----
                    with tc.tile_pool(name="ln1", bufs=1) as sb:
                        xTs = sb.tile([128, ET, T], BF16, name="xTs")
                        nc.sync.dma_start(
                            out=xTs,
                            in_=xT.rearrange("(i p) t -> p i t", p=128))
                        mu = sb.tile([1, T], F32, name="mu")
                        rstd = sb.tile([1, T], F32, name="rstd")
                        msr = sb.tile([1, T], F32, name="msr")
                        with tc.tile_pool(name="st1", bufs=1,
                                          space="PSUM") as ps:
                            for half in range(4):
                                hs = ds(512 * half, 512)
                                xsq = sb.tile([128, ET, 512], BF16,
                                              tag="xsq", bufs=1)
                                nc.vector.tensor_tensor(
                                    xsq, xTs[:, :, hs], xTs[:, :, hs], MUL)
                                psm = ps.tile([1, 512], F32, tag="psm",
                                              bufs=2)
                                psq = ps.tile([1, 512], F32, tag="psq",
                                              bufs=2)
                                for i in range(ET):
                                    nc.tensor.matmul(
                                        psm, ones_bf, xTs[:, i, hs],
                                        start=(i == 0), stop=(i == 7))
                                for i in range(ET):
                                    nc.tensor.matmul(
                                        psq, ones_bf, xsq[:, i, :],
                                        start=(i == 0), stop=(i == 7))
                                nc.vector.tensor_scalar_mul(
                                    mu[:, hs], psm, 1.0 / E)
                                nc.vector.tensor_scalar_mul(
                                    rstd[:, hs], psq, 1.0 / E)
                        # var = sq - mu^2 ; rstd = 1/sqrt(var+eps)
                        nc.vector.tensor_tensor(msr, mu, mu, MUL)
                        nc.vector.tensor_tensor(rstd, rstd, msr, SUB)
                        nc.scalar.activation(rstd, rstd, SQRT, bias=eps_sb)
                        nc.vector.reciprocal(rstd, rstd)
                        nc.vector.tensor_tensor(msr, mu, rstd, MUL)
                        # normalize in place: h = x*rstd - mu*rstd
                        nc.vector.tensor_tensor(
                            xTs, xTs, _bc3(nc, sb, rstd, ET, T, "bc1"), MUL)
                        nc.vector.tensor_tensor(
                            xTs, xTs, _bc3(nc, sb, msr, ET, T, "bc1"), SUB)

                        # ---- QKV ----
                        with tc.tile_pool(name="qkv", bufs=1,
                                          space="PSUM") as ps:
                          if skip_qkv:
                            nc.vector.memset(QT, 0.001)
                            nc.vector.memset(KT, 0.001)
                            nc.vector.memset(Vp[:, :, :, 0:64], 0.001)
                          else:
                              for w_dram, o_sb in ((qw, QT), (kw, KT)):
                                  w_sb = sb.tile([128, ET, HD], BF16,
                                                 tag="wqkv", bufs=1,
                                                 name="w_sb")
                                  nc.sync.dma_start(
                                      out=w_sb,
                                      in_=w_dram.rearrange("(i p) f -> p i f",
                                                           p=128))
                                  for m in range(4):
                                      pq = ps.tile([128, 4, 512], F32,
                                                   tag="pq", bufs=1, name="pq")
                                      for i in range(ET):
                                          for c in range(4):
                                              nc.tensor.matmul(
                                                  pq[:, c, :],
                                                  w_sb[:, i, ts(m, 128)],
                                                  xTs[:, i, ts(c, 512)],
                                                  start=(i == 0), stop=(i == 7))
                                      nc.vector.tensor_copy(
                                          o_sb[:, m, :].rearrange(
                                              "p (c q) -> p c q", c=4), pq)
                              vws = sb.tile([128, ET, HD], BF16,
                                            tag="wqkv", bufs=1, name="vws")
                              nc.sync.dma_start(
                                  out=vws,
                                  in_=vw.rearrange("(i p) f -> p i f", p=128))
                              for mt0 in range(0, KT_N, 2):
                                  pv = ps.tile([128, 2, 512], F32, tag="pv",
                                               bufs=2)
                                  for z in range(2):
                                      for i in range(ET):
                                          nc.tensor.matmul(
                                              pv[:, z, :],
                                              xTs[:, i, ts(mt0 + z, 128)],
                                              vws[:, i, :],
                                              start=(i == 0), stop=(i == 7))
                                  nc.vector.tensor_copy(
                                      Vp[:, mt0:mt0 + 2, :, 0:64],
                                      pv.rearrange("p t (h d) -> p t h d",
                                                   h=HPC))

                    # ---- A2: attention, j-outer for Ldweights sharing ----
                    with tc.tile_pool(name="att", bufs=1) as sb, \
                         tc.tile_pool(name="att_ps", bufs=1,
                                      space="PSUM") as ps:
                        if skip_attn:
                            nc.vector.memset(AO, 0.001)
                        for h in ([] if skip_attn else range(HPC)):
                            hp, z = h // 2, h % 2
                            pp = slice(64 * z, 64 * z + 64)
                            psO = ps.tile([65, 4, 512], F32, tag="psO",
                                          bufs=1, name="psO")
                            for j in range(KT_N):
                                cs = list(range(j // 4, 4))
                                nv = len(cs)
                                jsl = ts(j, 128)
                                pS = ps.tile([128, 4, 512], F32, tag="pS",
                                             bufs=1, name="pS")
                                for idx, c in enumerate(cs):
                                    nc.tensor.matmul(
                                        pS[:, idx, :], KT[pp, hp, jsl],
                                        QT[pp, hp, ts(c, 512)],
                                        start=True, stop=True)
                                PT = sb.tile([128, 4, 512], BF16, tag="PT",
                                             bufs=2, name="PT")
                                nc.scalar.activation(
                                    PT[:, 0:nv, :], pS[:, 0:nv, :], EXP,
                                    scale=float(HS) ** -0.5)
                                nc.vector.tensor_tensor(
                                    PT[:, 0, :], PT[:, 0, :],
                                    masks[:, j % 4, :], MUL)
                                for idx, c in enumerate(cs):
                                    nc.tensor.matmul(
                                        psO[:, c, :], Vp[:, j, h, :],
                                        PT[:, idx, :],
                                        start=(j == 0),
                                        stop=(j == 4 * c + 3))
                            rl = sb.tile([1, 4, 512], F32, tag="rl", bufs=2)
                            nc.vector.reciprocal(rl, psO[64:65, :, :])
                            rlb = sb.tile([64, 4, 512], F32, tag="rlb",
                                          bufs=2)
                            nc.gpsimd.partition_broadcast(rlb, rl)
                            nc.vector.tensor_tensor(
                                AO[pp, hp, :].rearrange(
                                    "p (c q) -> p c q", c=4),
                                psO[0:64, :, :], rlb, MUL)

                # ---- A3: attn projection (e-major) + reduce-scatter ----
                with tc.tile_pool(name="prj", bufs=1) as sb, \
                     tc.tile_pool(name="prj_ps", bufs=1, space="PSUM") as ps:
                    apws = sb.tile([128, 4, E], BF16, name="apws")
                    nc.sync.dma_start(
                        out=apws, in_=apw.rearrange("(k p) e -> p k e",
                                                    p=128))
                    for em in range(ET):
                        pP = [ps.tile([128, 1024], F32, tag=f"pP{z}", bufs=1,
                                      name=f"pP{z}") for z in range(2)]
                        for kh in range(4):
                            for ct in range(4):
                                nc.tensor.matmul(
                                    pP[ct // 2][:, ts(ct % 2, 512)],
                                    apws[:, kh, ts(em, 128)],
                                    AO[:, kh, ts(ct, 512)],
                                    start=(kh == 0), stop=(kh == 3))
                        for half in range(2):
                            po = sb.tile([128, 1024], F32, tag="po", bufs=2)
                            nc.vector.tensor_copy(po, pP[half])
                            nc.sync.dma_start(
                                out=part[half, ts(em, 128), :], in_=po)
                if single or no_cc:
                    nc.sync.dma_start(out=rs[:], in_=part[0, :, :])
                else:
                    nc.gpsimd.collective_compute(
                        "ReduceScatter", ADD, replica_groups=groups,
                        ins=[part[:]], outs=[rs[:]])
                # xp = rs + (x_half + attn_bias), e-major
                with tc.tile_pool(name="res", bufs=1) as sb:
                    xrs = sb.tile([128, ET, TH], F32, name="xrs")
                    nc.sync.dma_start(
                        out=xrs, in_=xrpT.rearrange("(i p) t -> p i t",
                                                    p=128))
                    rsb = sb.tile([128, ET, TH], F32, name="rsb")
                    nc.sync.dma_start(
                        out=rsb, in_=rs.rearrange("(j p) t -> p j t", p=128))
                    nc.vector.tensor_tensor(xp, rsb, xrs, ADD)

            # ======== phase B: LN2 + FFN (e-major) ========
            with tc.tile_pool(name="persB", bufs=1) as pB:
                h2T = pB.tile([128, ET, TH], BF16, name="h2T")
                ffh = pB.tile([128, FF // 128, TH], BF16, name="ffh")
                with tc.tile_pool(name="ln2", bufs=1) as sb, \
                     tc.tile_pool(name="ln2_ps", bufs=1, space="PSUM") as ps:
                    xpb = sb.tile([128, ET, TH], BF16, name="xpb")
                    nc.vector.tensor_copy(xpb, xp)
                    xqb = sb.tile([128, ET, TH], BF16, name="xqb")
                    nc.vector.tensor_tensor(xqb, xpb, xpb, MUL)
                    psm = ps.tile([1, 1024], F32, name="psm")
                    psq = ps.tile([1, 1024], F32, name="psq")
                    for i in range(ET):
                        for c in range(2):
                            nc.tensor.matmul(psm[:, ts(c, 512)], ones_bf,
                                             xpb[:, i, ts(c, 512)],
                                             start=(i == 0), stop=(i == 7))
                    for i in range(ET):
                        for c in range(2):
                            nc.tensor.matmul(psq[:, ts(c, 512)], ones_bf,
                                             xqb[:, i, ts(c, 512)],
                                             start=(i == 0), stop=(i == 7))
                    mu = sb.tile([1, TH], F32, name="mu2")
                    nc.vector.tensor_scalar_mul(mu, psm, 1.0 / E)
                    rstd = sb.tile([1, TH], F32, name="rstd2")
                    nc.vector.tensor_scalar_mul(rstd, psq, 1.0 / E)
                    msq = sb.tile([1, TH], F32, name="msq2")
                    nc.vector.tensor_tensor(msq, mu, mu, MUL)
                    nc.vector.tensor_tensor(rstd, rstd, msq, SUB)
                    nc.scalar.activation(rstd, rstd, SQRT, bias=eps_sb)
                    nc.vector.reciprocal(rstd, rstd)
                    msr = sb.tile([1, TH], F32, name="msr2")
                    nc.vector.tensor_tensor(msr, mu, rstd, MUL)
                    nc.vector.tensor_tensor(
                        h2T, xpb, _bc3(nc, sb, rstd, ET, TH, "bc2"), MUL)
                    nc.vector.tensor_tensor(
                        h2T, h2T, _bc3(nc, sb, msr, ET, TH, "bc2"), SUB)

                # ---- ff1 ----
                with tc.tile_pool(name="ff1", bufs=1) as sb, \
                     tc.tile_pool(name="ff1_ps", bufs=1, space="PSUM") as ps:
                    if skip_ffn:
                        nc.vector.memset(ffh, 0.001)
                    for half in ([] if skip_ffn else range(2)):
                        w1h = sb.tile([128, ET, 2048], BF16, tag="w1h",
                                      bufs=1)
                        nc.sync.dma_start(
                            out=w1h,
                            in_=fw1.rearrange("(i p) f -> p i f",
                                              p=128)[:, :,
                                                     ds(2048 * half, 2048)])
                        for m in range(16):
                            mf = 16 * half + m
                            pF = ps.tile([128, 1024], F32, tag="pF", bufs=2)
                            for i in range(ET):
                                for n in range(2):
                                    nc.tensor.matmul(
                                        pF[:, ts(n, 512)],
                                        w1h[:, i, ts(m, 128)],
                                        h2T[:, i, ts(n, 512)],
                                        start=(i == 0), stop=(i == 7))
                            nc.scalar.activation(ffh[:, mf, :], pF, RELU,
                                                 bias=fb1_sb[:, mf:mf + 1])

                # ---- ff2 + residual + out ----
                with tc.tile_pool(name="ff2", bufs=1) as sb, \
                     tc.tile_pool(name="ff2_ps", bufs=1, space="PSUM") as ps:
                    fw2s = sb.tile([128, FF // 128, E], BF16, name="fw2s")
                    nc.sync.dma_start(
                        out=fw2s, in_=fw2.rearrange("(k p) e -> p k e",
                                                    p=128))
                    for m in ([] if skip_ffn else range(ET)):
                        pG = ps.tile([128, 1024], F32, tag="pG", bufs=2)
                        for k in range(FF // 128):
                            for n in range(2):
                                nc.tensor.matmul(
                                    pG[:, ts(n, 512)],
                                    fw2s[:, k, ts(m, 128)],
                                    ffh[:, k, ts(n, 512)],
                                    start=(k == 0), stop=(k == 31))
                        fin = sb.tile([128, 1024], F32, tag="fin", bufs=2)
                        nc.vector.tensor_scalar_add(fin, pG,
                                                    fb2_sb[:, m:m + 1])
                        nc.vector.tensor_tensor(xp[:, m, :], fin,
                                                xp[:, m, :], ADD)
                    nc.sync.dma_start(
                        out=out.rearrange("(i p) t -> p i t", p=128),
                        in_=xp)

    with tile.TileContext(nc) as tc:
        for _ in range(loop_n):
            _emit(tc)
        if tok is not None:
            nc.sync.dma_start(out=tok, in_=out[0:1, 0:128])

    nc.compile()
    return nc


_CACHED = {}


def _prepare_inputs(x, qkv_w, attn_proj_w, attn_proj_b, ln1_g, ln1_b,
                    ln2_g, ln2_b, ff_w1, ff_b1, ff_w2, ff_b2):
    """Fold LN affine params into the weights, shard, and cast to bf16."""
    x = np.asarray(x, np.float32)
    qkv_w = np.asarray(qkv_w, np.float32) * np.asarray(ln1_g, np.float32)[:, None]
    qkv_b = np.asarray(ln1_b, np.float32) @ qkv_w  # [3*H*HS]
    assert np.abs(qkv_b).max() == 0.0, "nonzero ln1_b not supported"
    ff_w1f = np.asarray(ff_w1, np.float32) * np.asarray(ln2_g, np.float32)[:, None]
    ff_b1f = np.asarray(ff_b1, np.float32) + np.asarray(ln2_b, np.float32) @ ff_w1f
    apb = np.asarray(attn_proj_b, np.float32)

    fw1_bf = ff_w1f.astype(NPBF16)
    fw2_bf = np.asarray(ff_w2, np.float32).astype(NPBF16)
    fb1_t = np.ascontiguousarray(ff_b1f.reshape(FF // 128, 128).T)
    fb2_t = np.ascontiguousarray(
        np.asarray(ff_b2, np.float32).reshape(ET, 128).T)
    apw_bf = np.asarray(attn_proj_w, np.float32).astype(NPBF16)

    in_maps = []
    for c in range(NCORES):
        b, hh = c // 2, c % 2
        hsl = slice(512 * hh, 512 * hh + 512)
        tsl = slice(TH * hh, TH * hh + TH)
        in_maps.append({
            "xT": np.ascontiguousarray(x[b].T).astype(NPBF16),
            "xrpT": np.ascontiguousarray((x[b, tsl] + apb[None, :]).T),
            "qw": np.ascontiguousarray(qkv_w[:, hsl]).astype(NPBF16),
            "kw": np.ascontiguousarray(qkv_w[:, H * HS:][:, hsl]).astype(NPBF16),
            "vw": np.ascontiguousarray(qkv_w[:, 2 * H * HS:][:, hsl]).astype(NPBF16),
            "apw": np.ascontiguousarray(apw_bf[hsl, :]),
            "fw1": fw1_bf,
            "fb1": fb1_t,
            "fw2": fw2_bf,
            "fb2": fb2_t,
        })
    return in_maps


def kernel(**inputs):
    if "nc" not in _CACHED:
        _CACHED["nc"] = build_program()
    nc = _CACHED["nc"]
    in_maps = _prepare_inputs(**inputs)
    res = run_bass_kernel_spmd(nc, in_maps, list(range(NCORES)))
    full = np.empty((B, T, E), np.float32)
    for c in range(NCORES):
        b, hh = c // 2, c % 2
        full[b, TH * hh:TH * hh + TH] = res.results[c]["out"].T
    return full



# revision 16
# speedup vs baseline: 1.4205x; 1.4205x over previous
"""Fused pre-LN transformer block (causal MHA + FFN) on 8 TRN2 NeuronCores.

Sharding: core c handles batch b = c//2 and head-half hh = c%2 (8 of 16 heads).
Attention runs fully local per (batch, head-half); the attention projection
produces a partial sum that is ReduceScattered (over token dim) within each
core pair, so FFN runs token-sharded (1024 tokens/core). Output per core is
its token slice, stored E-major [E, TH]; the host transposes when gathering.

v3: latency-focused restructure of v2:
- LN1+QKV pipelined over 4 token chunks (stats/normalize of chunk c overlap
  QKV matmuls of neighboring chunks); rstd via scalar-engine Rsqrt.
- Attention q-chunk-column-outer (all 8 heads per column) with a single-bank
  psO accumulator (bufs=2): softmax denominators are staged per column into
  an 8-partition tile and reciprocated in ONE 3.2us DVE op for all heads;
  normalization is applied lazily in-place on the raw attention output.
- Attention projection runs per column as soon as the column is normalized;
  the two bf16 pair ReduceScatters are issued mid-attention and fully hide
  behind the remaining columns / LN2 / ff1.
- FFN: fw1 streamed in 1MB eighths (re-fetched per token half), fw2 in
  quarters prefetched during ff1; relu + biases fused on the scalar engine.
"""

import numpy as np
import ml_dtypes

import concourse.bass as bass
import concourse.mybir as mybir
import concourse.tile as tile
from concourse import bacc
from concourse.bass import ts, ds
from concourse.bass_utils import run_bass_kernel_spmd

BF16 = mybir.dt.bfloat16
F32 = mybir.dt.float32
NPBF16 = ml_dtypes.bfloat16

B, T, E = 4, 2048, 1024
H, HS = 16, 64
FF = 4 * E
EPS = 1e-5
NCORES = 8
HPC = 8            # heads per core
HD = HPC * HS      # 512 head dims per core
TH = T // 2        # 1024 tokens per core for FFN
KT_N = T // 128    # 16 k-tiles
ET = E // 128      # 8 e-tiles
ADD = mybir.AluOpType.add
SUB = mybir.AluOpType.subtract
MUL = mybir.AluOpType.mult
EXP = mybir.ActivationFunctionType.Exp
RELU = mybir.ActivationFunctionType.Relu
SQRT = mybir.ActivationFunctionType.Sqrt
COPY = mybir.ActivationFunctionType.Copy


def build_program(single=False, dbg=False):
    nc = bacc.Bacc("TRN2", target_bir_lowering=False, debug=False,
                   num_devices=1 if single else NCORES)
    dbg_t = {}
    if dbg:
        for name, shape, dt in (
                ("dXN", [128, ET, T], BF16), ("dQT", [128, 4, T], BF16),
                ("dKT", [128, 4, T], BF16),
                ("dVp", [128, KT_N, HPC, 65], BF16),
                ("dAO", [128, 4, T], BF16), ("dLA", [97, 4, 512], F32),
                ("dLB", [97, 4, 512], F32), ("drsA", [E, 512], BF16),
                ("drsB", [E, 512], BF16), ("dxp", [128, ET, TH], F32),
                ("dh2", [128, ET, TH], BF16)):
            dbg_t[name] = nc.dram_tensor(name, shape, dt,
                                         kind="ExternalOutput").ap()

    # ---- I/O ----
    xT = nc.dram_tensor("xT", [E, T], BF16, kind="ExternalInput").ap()
    xrpT = nc.dram_tensor("xrpT", [E, TH], F32, kind="ExternalInput").ap()
    qw = nc.dram_tensor("qw", [E, HD], BF16, kind="ExternalInput").ap()
    kw = nc.dram_tensor("kw", [E, HD], BF16, kind="ExternalInput").ap()
    vw = nc.dram_tensor("vw", [E, HD], BF16, kind="ExternalInput").ap()
    apw = nc.dram_tensor("apw", [HD, E], BF16, kind="ExternalInput").ap()
    fw1 = nc.dram_tensor("fw1", [E, FF], BF16, kind="ExternalInput").ap()
    fb1 = nc.dram_tensor("fb1", [128, FF // 128], F32,
                         kind="ExternalInput").ap()
    fw2 = nc.dram_tensor("fw2", [FF, E], BF16, kind="ExternalInput").ap()
    fb2 = nc.dram_tensor("fb2", [128, ET], F32, kind="ExternalInput").ap()
    out = nc.dram_tensor("out", [E, TH], F32, kind="ExternalOutput").ap()

    # internal DRAM for the two chunked pair-wise reduce-scatters (bf16).
    # partA holds global token quarters {0, 2}, partB {1, 3}; RS over the
    # core pair scatters dim0, so the even core gets quarters 0,1 and the
    # odd core quarters 2,3 == its own token half.
    partA = nc.dram_tensor("partA", [2, E, 512], BF16).ap()
    partB = nc.dram_tensor("partB", [2, E, 512], BF16).ap()
    rsA = nc.dram_tensor("rsA", [E, 512], BF16).ap()
    rsB = nc.dram_tensor("rsB", [E, 512], BF16).ap()
    groups = [[0, 1], [2, 3], [4, 5], [6, 7]]

    def _emit(tc):
        with tc.tile_pool(name="const", bufs=1) as constp:
            ones_bf = constp.tile([128, 1], BF16)
            nc.gpsimd.memset(ones_bf, 1.0)
            eps_sb = constp.tile([1, 1], F32)
            nc.gpsimd.memset(eps_sb, EPS)
            scratch1 = constp.tile([1, 64], F32)
            nc.gpsimd.memset(scratch1, 1.0)
            fb1_sb = constp.tile([128, FF // 128], F32)
            nc.sync.dma_start(out=fb1_sb, in_=fb1)
            fb2_sb = constp.tile([128, ET], F32)
            nc.sync.dma_start(out=fb2_sb, in_=fb2)

            # x' (post-attention residual input), lives to the end.
            xp = constp.tile([128, ET, TH], F32, name="xp")

            # ======== phase A: attention ========
            with tc.tile_pool(name="persA", bufs=1) as pA:
                QT = pA.tile([128, 4, T], BF16, name="QT")
                KT = pA.tile([128, 4, T], BF16, name="KT")
                AO = pA.tile([128, 4, T], BF16, name="AO")
                Vp = pA.tile([128, KT_N, HPC, 65], BF16, name="Vp")
                nc.vector.memset(Vp[:, :, :, 64:65], 1.0)
                # softmax denominators: 4 heads per tile on partitions
                # {0,32,64,96} (DVE partition-start constraint), so the
                # per-column reciprocal batches 4 heads per op.
                LstA = pA.tile([97, 4, 512], F32, name="LstA")
                LstB = pA.tile([97, 4, 512], F32, name="LstB")
                nc.vector.memset(LstA, 1.0)
                nc.vector.memset(LstB, 1.0)
                # causal masks for the 4 diagonal k-offsets:
                # masks[p, kk, qq] = 1 if qq >= 128*kk + p else 0
                masks = pA.tile([128, 4, 512], BF16, name="masks")
                nc.gpsimd.memset(masks, 1.0)
                nc.gpsimd.affine_select(
                    out=masks, in_=masks, compare_op=mybir.AluOpType.is_ge,
                    fill=0.0, base=0, pattern=[[-128, 4], [1, 512]],
                    channel_multiplier=-1)

                # ---- A1: LN1 + QKV, pipelined over 4 token chunks ----
                with tc.tile_pool(name="ln1", bufs=1) as sb, \
                     tc.tile_pool(name="ln1_ps", bufs=1, space="PSUM") as ps:
                    xTs = sb.tile([128, ET, T], BF16, name="xTs")
                    w_q = sb.tile([128, ET, HD], BF16, name="w_q")
                    w_k = sb.tile([128, ET, HD], BF16, name="w_k")
                    w_v = sb.tile([128, ET, HD], BF16, name="w_v")
                    nc.sync.dma_start(
                        out=w_q, in_=qw.rearrange("(i p) f -> p i f", p=128))
                    nc.sync.dma_start(
                        out=w_k, in_=kw.rearrange("(i p) f -> p i f", p=128))
                    nc.sync.dma_start(
                        out=w_v, in_=vw.rearrange("(i p) f -> p i f", p=128))
                    for c in range(4):
                        csl = ds(512 * c, 512)
                        nc.sync.dma_start(
                            out=xTs[:, :, csl],
                            in_=xT.rearrange("(i p) t -> p i t",
                                             p=128)[:, :, csl])
                        # stats
                        xsq = sb.tile([128, ET, 512], BF16, tag="xsq",
                                      bufs=1)
                        nc.vector.tensor_tensor(
                            xsq, xTs[:, :, csl], xTs[:, :, csl], MUL)
                        psm = ps.tile([1, 512], F32, tag="psm", bufs=2)
                        psq = ps.tile([1, 512], F32, tag="psq", bufs=2)
                        for i in range(ET):
                            nc.tensor.matmul(
                                psm, ones_bf, xTs[:, i, csl],
                                start=(i == 0), stop=(i == 7))
                        for i in range(ET):
                            nc.tensor.matmul(
                                psq, ones_bf, xsq[:, i, :],
                                start=(i == 0), stop=(i == 7))
                        mu = sb.tile([1, 512], F32, tag="mu", bufs=2)
                        rstd = sb.tile([1, 512], F32, tag="rstd", bufs=2)
                        msr = sb.tile([1, 512], F32, tag="msr", bufs=2)
                        nc.vector.tensor_scalar_mul(mu, psm, 1.0 / E)
                        nc.vector.tensor_scalar_mul(rstd, psq, 1.0 / E)
                        nc.vector.tensor_tensor(msr, mu, mu, MUL)
                        nc.vector.tensor_tensor(rstd, rstd, msr, SUB)
                        # rstd = 1/sqrt(var + eps)
                        nc.scalar.activation(rstd, rstd, SQRT, bias=eps_sb)
                        nc.vector.reciprocal(rstd, rstd)
                        nc.vector.tensor_tensor(msr, mu, rstd, MUL)
                        bc_rs = sb.tile([128, 512], F32, tag="bc_rs", bufs=2)
                        nc.gpsimd.partition_broadcast(bc_rs, rstd)
                        bc_ms = sb.tile([128, 512], F32, tag="bc_ms", bufs=2)
                        nc.gpsimd.partition_broadcast(bc_ms, msr)
                        # normalize in place: h = x*rstd - mu*rstd
                        nc.vector.tensor_tensor(
                            xTs[:, :, csl], xTs[:, :, csl],
                            bc_rs.unsqueeze(1).broadcast_to([128, ET, 512]),
                            MUL)
                        nc.vector.tensor_tensor(
                            xTs[:, :, csl], xTs[:, :, csl],
                            bc_ms.unsqueeze(1).broadcast_to([128, ET, 512]),
                            SUB)
                        # QKV for this chunk
                        for wi, (w_sb, o_sb) in enumerate(
                                ((w_q, QT), (w_k, KT))):
                            for m in range(4):
                                pq = ps.tile([128, 512], F32, tag="pq",
                                             bufs=2)
                                for i in range(ET):
                                    nc.tensor.matmul(
                                        pq, w_sb[:, i, ts(m, 128)],
                                        xTs[:, i, csl],
                                        start=(i == 0), stop=(i == 7))
                                if (wi * 4 + m) % 2 == 0:
                                    nc.vector.tensor_copy(
                                        o_sb[:, m, csl], pq)
                                else:
                                    nc.scalar.activation(
                                        o_sb[:, m, csl], pq, COPY)
                        for mt in range(4):
                            kt = 4 * c + mt
                            pv = ps.tile([128, 512], F32, tag="pv", bufs=2)
                            for i in range(ET):
                                nc.tensor.matmul(
                                    pv, xTs[:, i, ts(kt, 128)], w_v[:, i, :],
                                    start=(i == 0), stop=(i == 7))
                            vdst = Vp[:, kt, :, 0:64]
                            pvr = pv.rearrange("p (h d) -> p h d", h=HPC)
                            if mt % 2 == 0:
                                nc.vector.tensor_copy(vdst, pvr)
                            else:
                                nc.scalar.activation(vdst, pvr, COPY)
                    # preload the exp table while QKV finishes
                    nc.scalar.activation(scratch1, scratch1, EXP)
                    if dbg:
                        nc.sync.dma_start(out=dbg_t["dXN"], in_=xTs)
                        nc.sync.dma_start(out=dbg_t["dQT"], in_=QT)
                        nc.sync.dma_start(out=dbg_t["dKT"], in_=KT)
                        nc.sync.dma_start(out=dbg_t["dVp"], in_=Vp)

                # ---- A2+A3: attention columns interleaved with proj/RS ----
                with tc.tile_pool(name="att", bufs=1) as sb, \
                     tc.tile_pool(name="att_ps", bufs=1,
                                  space="PSUM") as ps:
                    apws = sb.tile([128, 4, E], BF16, name="apws")
                    nc.sync.dma_start(
                        out=apws, in_=apw.rearrange("(k p) e -> p k e",
                                                    p=128))
                    xrs = sb.tile([128, ET, TH], F32, name="xrs")
                    nc.sync.dma_start(
                        out=xrs, in_=xrpT.rearrange("(i p) t -> p i t",
                                                    p=128))
                    for c in range(4):
                        csl = ts(c, 512)
                        for h in range(HPC):
                            hp, z = h // 2, h % 2
                            pp = slice(64 * z, 64 * z + 64)
                            psO = ps.tile([65, 512], F32, tag="psO",
                                          bufs=2, name="psO")
                            for jg in range(c + 1):
                                pS4 = ps.tile([128, 4, 512], F32, tag="pS",
                                              bufs=1, name="pS4")
                                for kk in range(4):
                                    nc.tensor.matmul(
                                        pS4[:, kk, :],
                                        KT[pp, hp, ts(4 * jg + kk, 128)],
                                        QT[pp, hp, csl],
                                        start=True, stop=True)
                                PT4 = sb.tile([128, 4, 512], BF16, tag="PT",
                                              bufs=2, name="PT4")
                                nc.scalar.activation(
                                    PT4, pS4, EXP, scale=float(HS) ** -0.5)
                                if jg == c:
                                    nc.vector.tensor_tensor(
                                        PT4, PT4, masks, MUL)
                                for kk in range(4):
                                    j = 4 * jg + kk
                                    nc.tensor.matmul(
                                        psO, Vp[:, j, h, :], PT4[:, kk, :],
                                        start=(j == 0), stop=(j == 4 * c + 3))
                            # stage raw output + denominator; normalize later
                            nc.vector.tensor_copy(
                                AO[pp, hp, csl], psO[0:64, :])
                            lst = LstA if h < 4 else LstB
                            lp = 32 * (h % 4)
                            nc.vector.tensor_copy(
                                lst[lp:lp + 1, c, :], psO[64:65, :])
                        # two reciprocals cover all 8 heads of this column
                        rclA = sb.tile([97, 512], F32, tag="rclA", bufs=2)
                        nc.vector.reciprocal(rclA, LstA[:, c, :])
                        rclB = sb.tile([97, 512], F32, tag="rclB", bufs=2)
                        nc.vector.reciprocal(rclB, LstB[:, c, :])
                        for h in range(HPC):
                            hp, z = h // 2, h % 2
                            pp = slice(64 * z, 64 * z + 64)
                            rcl = rclA if h < 4 else rclB
                            lp = 32 * (h % 4)
                            # HW partition_broadcast reads the tile's
                            # partition 0 only -> bounce through a p0 row.
                            t0 = sb.tile([1, 512], F32, tag="t0", bufs=3)
                            nc.vector.tensor_copy(t0, rcl[lp:lp + 1, :])
                            rbc = sb.tile([128, 512], F32, tag="rbc", bufs=3)
                            nc.gpsimd.partition_broadcast(rbc, t0)
                            nc.vector.tensor_tensor(
                                AO[pp, hp, csl], AO[pp, hp, csl], rbc[pp, :],
                                MUL)
                        # projection for this column -> bf16 partials
                        dst = partA if c % 2 == 0 else partB
                        slot = c // 2
                        for em in range(ET):
                            pP = ps.tile([128, 512], F32, tag="pP", bufs=2)
                            for kh in range(4):
                                nc.tensor.matmul(
                                    pP, apws[:, kh, ts(em, 128)],
                                    AO[:, kh, csl],
                                    start=(kh == 0), stop=(kh == 3))
                            po = sb.tile([128, 512], BF16, tag="po", bufs=4)
                            if em % 2 == 0:
                                nc.vector.tensor_copy(po, pP)
                            else:
                                nc.scalar.activation(po, pP, COPY)
                            nc.sync.dma_start(
                                out=dst[slot, ts(em, 128), :], in_=po)
                        if c == 2:
                            if single:
                                nc.sync.dma_start(out=rsA[:],
                                                  in_=partA[0, :, :])
                            else:
                                nc.gpsimd.collective_compute(
                                    "ReduceScatter", ADD,
                                    replica_groups=groups,
                                    ins=[partA[:]], outs=[rsA[:]])
                        if c == 3:
                            if single:
                                nc.sync.dma_start(out=rsB[:],
                                                  in_=partB[0, :, :])
                            else:
                                nc.gpsimd.collective_compute(
                                    "ReduceScatter", ADD,
                                    replica_groups=groups,
                                    ins=[partB[:]], outs=[rsB[:]])
                    if dbg:
                        nc.sync.dma_start(out=dbg_t["dAO"], in_=AO)
                        nc.sync.dma_start(out=dbg_t["dLA"], in_=LstA)
                        nc.sync.dma_start(out=dbg_t["dLB"], in_=LstB)
                        nc.sync.dma_start(out=dbg_t["drsA"], in_=rsA[:])
                        nc.sync.dma_start(out=dbg_t["drsB"], in_=rsB[:])
                    # xp = rs + (x_half + attn_bias), per token half
                    for half, rsx in enumerate((rsA, rsB)):
                        hsl = ds(512 * half, 512)
                        rsb = sb.tile([128, ET, 512], BF16, tag="rsb",
                                      bufs=2)
                        nc.sync.dma_start(
                            out=rsb,
                            in_=rsx.rearrange("(j p) t -> p j t", p=128))
                        nc.vector.tensor_tensor(
                            xp[:, :, hsl], rsb, xrs[:, :, hsl], ADD)

            # ======== phase B: LN2 + FFN (e-major), token-halved ========
            with tc.tile_pool(name="persB", bufs=1) as pB:
                if dbg:
                    nc.sync.dma_start(out=dbg_t["dxp"], in_=xp)
                h2T = pB.tile([128, ET, TH], BF16, name="h2T")
                ffh = pB.tile([128, FF // 128, TH], BF16, name="ffh")
                with tc.tile_pool(name="ffw", bufs=1) as sbw, \
                     tc.tile_pool(name="ff_ps", bufs=1, space="PSUM") as ps:
                    # LN2 + ff1 per token half
                    for half in range(2):
                        hsl = ds(512 * half, 512)
                        xpb = sbw.tile([128, ET, 512], BF16, tag="xpb",
                                       bufs=1)
                        nc.scalar.activation(xpb, xp[:, :, hsl], COPY)
                        xqb = sbw.tile([128, ET, 512], BF16, tag="xqb",
                                       bufs=1)
                        nc.vector.tensor_tensor(xqb, xpb, xpb, MUL)
                        psm = ps.tile([1, 512], F32, tag="psm2", bufs=1)
                        psq = ps.tile([1, 512], F32, tag="psq2", bufs=1)
                        for i in range(ET):
                            nc.tensor.matmul(
                                psm, ones_bf, xpb[:, i, :],
                                start=(i == 0), stop=(i == 7))
                        for i in range(ET):
                            nc.tensor.matmul(
                                psq, ones_bf, xqb[:, i, :],
                                start=(i == 0), stop=(i == 7))
                        mu = sbw.tile([1, 512], F32, tag="mu2", bufs=1)
                        rstd = sbw.tile([1, 512], F32, tag="rstd2", bufs=1)
                        msr = sbw.tile([1, 512], F32, tag="msr2", bufs=1)
                        nc.vector.tensor_scalar_mul(mu, psm, 1.0 / E)
                        nc.vector.tensor_scalar_mul(rstd, psq, 1.0 / E)
                        nc.vector.tensor_tensor(msr, mu, mu, MUL)
                        nc.vector.tensor_tensor(rstd, rstd, msr, SUB)
                        nc.scalar.activation(rstd, rstd, SQRT, bias=eps_sb)
                        nc.vector.reciprocal(rstd, rstd)
                        nc.vector.tensor_tensor(msr, mu, rstd, MUL)
                        bc_rs = sbw.tile([128, 512], F32, tag="bc_rs2",
                                         bufs=1)
                        nc.gpsimd.partition_broadcast(bc_rs, rstd)
                        bc_ms = sbw.tile([128, 512], F32, tag="bc_ms2",
                                         bufs=1)
                        nc.gpsimd.partition_broadcast(bc_ms, msr)
                        nc.vector.tensor_tensor(
                            h2T[:, :, hsl], xpb,
                            bc_rs.unsqueeze(1).broadcast_to([128, ET, 512]),
                            MUL)
                        nc.vector.tensor_tensor(
                            h2T[:, :, hsl], h2T[:, :, hsl],
                            bc_ms.unsqueeze(1).broadcast_to([128, ET, 512]),
                            SUB)
                        # ---- ff1 for this token half ----
                        for m in range(FF // 128):
                            if m % 4 == 0:
                                w1e = sbw.tile([128, ET, 512], BF16,
                                               tag="w1e", bufs=2)
                                nc.sync.dma_start(
                                    out=w1e,
                                    in_=fw1.rearrange(
                                        "(i p) f -> p i f",
                                        p=128)[:, :, ds(512 * (m // 4), 512)])
                            pF = ps.tile([128, 512], F32, tag="pF", bufs=3)
                            for i in range(ET):
                                nc.tensor.matmul(
                                    pF, w1e[:, i, ts(m % 4, 128)],
                                    h2T[:, i, hsl],
                                    start=(i == 0), stop=(i == 7))
                            nc.scalar.activation(
                                ffh[:, m, hsl], pF, RELU,
                                bias=fb1_sb[:, m:m + 1])
                    if dbg:
                        nc.sync.dma_start(out=dbg_t["dh2"], in_=h2T)
                    # ---- ff2 + residual + out, E-quarters x token halves ----
                    for eq in range(4):
                        w2q = sbw.tile([128, FF // 128, 256], BF16,
                                       tag="w2q", bufs=2)
                        nc.sync.dma_start(
                            out=w2q,
                            in_=fw2.rearrange("(k p) e -> p k e",
                                              p=128)[:, :,
                                                     ds(256 * eq, 256)])
                        for half in range(2):
                            hsl = ds(512 * half, 512)
                            for m2 in range(2):
                                m = 2 * eq + m2
                                pG = ps.tile([128, 512], F32, tag="pG",
                                             bufs=2)
                                for k in range(FF // 128):
                                    nc.tensor.matmul(
                                        pG, w2q[:, k, ts(m2, 128)],
                                        ffh[:, k, hsl],
                                        start=(k == 0), stop=(k == 31))
                                fin = sbw.tile([128, 512], F32, tag="fin",
                                               bufs=3)
                                nc.vector.tensor_tensor(
                                    fin, pG, xp[:, m, hsl], ADD)
                                nc.scalar.activation(
                                    fin, fin,
                                    mybir.ActivationFunctionType.Identity,
                                    bias=fb2_sb[:, m:m + 1])
                                nc.sync.dma_start(
                                    out=out.rearrange(
                                        "(i p) t -> p i t", p=128)[:, m, hsl],
                                    in_=fin)

    with tile.TileContext(nc) as tc:
        _emit(tc)

    nc.compile()
    return nc


_CACHED = {}


def _prepare_inputs(x, qkv_w, attn_proj_w, attn_proj_b, ln1_g, ln1_b,
                    ln2_g, ln2_b, ff_w1, ff_b1, ff_w2, ff_b2):
    """Fold LN affine params into the weights, shard, and cast to bf16."""
    x = np.asarray(x, np.float32)
    qkv_w = np.asarray(qkv_w, np.float32) * np.asarray(ln1_g, np.float32)[:, None]
    qkv_b = np.asarray(ln1_b, np.float32) @ qkv_w  # [3*H*HS]
    assert np.abs(qkv_b).max() == 0.0, "nonzero ln1_b not supported"
    ff_w1f = np.asarray(ff_w1, np.float32) * np.asarray(ln2_g, np.float32)[:, None]
    ff_b1f = np.asarray(ff_b1, np.float32) + np.asarray(ln2_b, np.float32) @ ff_w1f
    apb = np.asarray(attn_proj_b, np.float32)

    fw1_bf = ff_w1f.astype(NPBF16)
    fw2_bf = np.asarray(ff_w2, np.float32).astype(NPBF16)
    fb1_t = np.ascontiguousarray(ff_b1f.reshape(FF // 128, 128).T)
    fb2_t = np.ascontiguousarray(
        np.asarray(ff_b2, np.float32).reshape(ET, 128).T)
    apw_bf = np.asarray(attn_proj_w, np.float32).astype(NPBF16)

    in_maps = []
    for c in range(NCORES):
        b, hh = c // 2, c % 2
        hsl = slice(512 * hh, 512 * hh + 512)
        tsl = slice(TH * hh, TH * hh + TH)
        in_maps.append({
            "xT": np.ascontiguousarray(x[b].T).astype(NPBF16),
            "xrpT": np.ascontiguousarray((x[b, tsl] + apb[None, :]).T),
            "qw": np.ascontiguousarray(qkv_w[:, hsl]).astype(NPBF16),
            "kw": np.ascontiguousarray(qkv_w[:, H * HS:][:, hsl]).astype(NPBF16),
            "vw": np.ascontiguousarray(qkv_w[:, 2 * H * HS:][:, hsl]).astype(NPBF16),
            "apw": np.ascontiguousarray(apw_bf[hsl, :]),
            "fw1": fw1_bf,
            "fb1": fb1_t,
            "fw2": fw2_bf,
            "fb2": fb2_t,
        })
    return in_maps


def kernel(**inputs):
    if "nc" not in _CACHED:
        _CACHED["nc"] = build_program()
    nc = _CACHED["nc"]
    in_maps = _prepare_inputs(**inputs)
    res = run_bass_kernel_spmd(nc, in_maps, list(range(NCORES)))
    full = np.empty((B, T, E), np.float32)
    for c in range(NCORES):
        b, hh = c // 2, c % 2
        full[b, TH * hh:TH * hh + TH] = res.results[c]["out"].T
    return full


# revision 21
# speedup vs baseline: 1.4758x; 1.0390x over previous
"""Fused pre-LN transformer block (causal MHA + FFN) on 8 TRN2 NeuronCores.

Sharding: core c handles batch b = c//2 and head-half hh = c%2 (8 of 16 heads).
Attention runs fully local per (batch, head-half); the attention projection
produces a partial sum that is ReduceScattered (over token dim) within each
core pair, so FFN runs token-sharded (1024 tokens/core). Output per core is
its token slice, stored E-major [E, TH]; the host transposes when gathering.

v3: latency-focused restructure of v2:
- LN1+QKV pipelined over 4 token chunks (stats/normalize of chunk c overlap
  QKV matmuls of neighboring chunks); rstd via scalar-engine Rsqrt.
- Attention q-chunk-column-outer (all 8 heads per column) with a single-bank
  psO accumulator (bufs=2): softmax denominators are staged per column into
  an 8-partition tile and reciprocated in ONE 3.2us DVE op for all heads;
  normalization is applied lazily in-place on the raw attention output.
- Attention projection runs per column as soon as the column is normalized;
  the two bf16 pair ReduceScatters are issued mid-attention and fully hide
  behind the remaining columns / LN2 / ff1.
- FFN: fw1 streamed in 1MB eighths (re-fetched per token half), fw2 in
  quarters prefetched during ff1; relu + biases fused on the scalar engine.
"""

import numpy as np
import ml_dtypes

import concourse.bass as bass
import concourse.mybir as mybir
import concourse.tile as tile
from concourse import bacc
from concourse.bass import ts, ds
from concourse.bass_utils import run_bass_kernel_spmd

BF16 = mybir.dt.bfloat16
F32 = mybir.dt.float32
NPBF16 = ml_dtypes.bfloat16

B, T, E = 4, 2048, 1024
H, HS = 16, 64
FF = 4 * E
EPS = 1e-5
NCORES = 8
HPC = 8            # heads per core
HD = HPC * HS      # 512 head dims per core
TH = T // 2        # 1024 tokens per core for FFN
KT_N = T // 128    # 16 k-tiles
ET = E // 128      # 8 e-tiles
ADD = mybir.AluOpType.add
SUB = mybir.AluOpType.subtract
MUL = mybir.AluOpType.mult
EXP = mybir.ActivationFunctionType.Exp
RELU = mybir.ActivationFunctionType.Relu
SQRT = mybir.ActivationFunctionType.Sqrt
COPY = mybir.ActivationFunctionType.Copy


def build_program(single=False, dbg=False):
    nc = bacc.Bacc("TRN2", target_bir_lowering=False, debug=False,
                   num_devices=1 if single else NCORES)
    dbg_t = {}
    if dbg:
        for name, shape, dt in (
                ("dXN", [128, ET, T], BF16), ("dQT", [128, 4, T], BF16),
                ("dKT", [128, 4, T], BF16),
                ("dVp", [128, KT_N, HPC, 65], BF16),
                ("dAO", [128, 4, T], BF16), ("dLA", [97, 4, 512], F32),
                ("dLB", [97, 4, 512], F32), ("drsA", [E, 512], BF16),
                ("drsB", [E, 512], BF16), ("dxp", [128, ET, TH], F32),
                ("dh2", [128, ET, TH], BF16)):
            dbg_t[name] = nc.dram_tensor(name, shape, dt,
                                         kind="ExternalOutput").ap()

    # ---- I/O ----
    xT = nc.dram_tensor("xT", [E, T], BF16, kind="ExternalInput").ap()
    xrpT = nc.dram_tensor("xrpT", [E, TH], F32, kind="ExternalInput").ap()
    qw = nc.dram_tensor("qw", [E, HD], BF16, kind="ExternalInput").ap()
    kw = nc.dram_tensor("kw", [E, HD], BF16, kind="ExternalInput").ap()
    vw = nc.dram_tensor("vw", [E, HD], BF16, kind="ExternalInput").ap()
    apw = nc.dram_tensor("apw", [HD, E], BF16, kind="ExternalInput").ap()
    fw1 = nc.dram_tensor("fw1", [E, FF], BF16, kind="ExternalInput").ap()
    fb1 = nc.dram_tensor("fb1", [128, FF // 128], F32,
                         kind="ExternalInput").ap()
    fw2 = nc.dram_tensor("fw2", [FF, E], BF16, kind="ExternalInput").ap()
    fb2 = nc.dram_tensor("fb2", [128, ET], F32, kind="ExternalInput").ap()
    out = nc.dram_tensor("out", [E, TH], F32, kind="ExternalOutput").ap()

    # internal DRAM for the two chunked pair-wise reduce-scatters (bf16).
    # partA holds global token quarters {0, 2}, partB {1, 3}; RS over the
    # core pair scatters dim0, so the even core gets quarters 0,1 and the
    # odd core quarters 2,3 == its own token half.
    partA = nc.dram_tensor("partA", [2, E, 512], BF16).ap()
    partB = nc.dram_tensor("partB", [2, E, 512], BF16).ap()
    rsA = nc.dram_tensor("rsA", [E, 512], BF16).ap()
    rsB = nc.dram_tensor("rsB", [E, 512], BF16).ap()
    groups = [[0, 1], [2, 3], [4, 5], [6, 7]]

    def _emit(tc):
        with tc.tile_pool(name="const", bufs=1) as constp:
            ones_bf = constp.tile([128, 1], BF16)
            nc.gpsimd.memset(ones_bf, 1.0)
            eps_sb = constp.tile([1, 1], F32)
            nc.gpsimd.memset(eps_sb, EPS)
            scratch1 = constp.tile([1, 64], F32)
            nc.gpsimd.memset(scratch1, 1.0)
            fb1_sb = constp.tile([128, FF // 128], F32)
            fb2_sb = constp.tile([128, ET], F32)

            # x' (post-attention residual input), lives to the end.
            xp = constp.tile([128, ET, TH], F32, name="xp")

            # ======== phase A: attention ========
            with tc.tile_pool(name="persA", bufs=1) as pA:
                QT = pA.tile([128, 4, T], BF16, name="QT")
                KT = pA.tile([128, 4, T], BF16, name="KT")
                AO = pA.tile([128, 4, T], BF16, name="AO")
                Vp = pA.tile([128, KT_N, HPC, 65], BF16, name="Vp")
                nc.vector.memset(Vp[:, :, :, 64:65], 1.0)
                # softmax denominators: 4 heads per tile on partitions
                # {0,32,64,96} (DVE partition-start constraint), so the
                # per-column reciprocal batches 4 heads per op.
                LstA = pA.tile([97, 4, 512], F32, name="LstA")
                LstB = pA.tile([97, 4, 512], F32, name="LstB")
                nc.vector.memset(LstA, 1.0)
                nc.vector.memset(LstB, 1.0)
                # causal masks for the 4 diagonal k-offsets:
                # masks[p, kk, qq] = 1 if qq >= 128*kk + p else 0
                masks = pA.tile([128, 4, 512], BF16, name="masks")
                nc.gpsimd.memset(masks, 1.0)
                nc.gpsimd.affine_select(
                    out=masks, in_=masks, compare_op=mybir.AluOpType.is_ge,
                    fill=0.0, base=0, pattern=[[-128, 4], [1, 512]],
                    channel_multiplier=-1)

                # ---- A1: LN1 + QKV, software-pipelined over 4 chunks ----
                with tc.tile_pool(name="ln1", bufs=1) as sb, \
                     tc.tile_pool(name="ln1_ps", bufs=1, space="PSUM") as ps:
                    xTs = sb.tile([128, ET, T], BF16, name="xTs")
                    w_q = sb.tile([128, ET, HD], BF16, name="w_q")
                    w_k = sb.tile([128, ET, HD], BF16, name="w_k")
                    w_v = sb.tile([128, ET, HD], BF16, name="w_v")
                    nc.sync.dma_start(
                        out=xTs[:, :, ds(0, 512)],
                        in_=xT.rearrange("(i p) t -> p i t",
                                         p=128)[:, :, ds(0, 512)])
                    nc.sync.dma_start(
                        out=w_q, in_=qw.rearrange("(i p) f -> p i f", p=128))
                    nc.sync.dma_start(
                        out=w_k, in_=kw.rearrange("(i p) f -> p i f", p=128))
                    nc.sync.dma_start(
                        out=w_v, in_=vw.rearrange("(i p) f -> p i f", p=128))

                    def emit_stats(c):
                        csl = ds(512 * c, 512)
                        if c > 0:
                            nc.sync.dma_start(
                                out=xTs[:, :, csl],
                                in_=xT.rearrange("(i p) t -> p i t",
                                                 p=128)[:, :, csl])
                        xsq = sb.tile([128, ET, 512], BF16, tag="xsq",
                                      bufs=2)
                        nc.vector.tensor_tensor(
                            xsq, xTs[:, :, csl], xTs[:, :, csl], MUL)
                        psm = ps.tile([1, 512], F32, tag="psm", bufs=2)
                        psq = ps.tile([1, 512], F32, tag="psq", bufs=2)
                        for i in range(ET):
                            nc.tensor.matmul(
                                psm, ones_bf, xTs[:, i, csl],
                                start=(i == 0), stop=(i == 7))
                        for i in range(ET):
                            nc.tensor.matmul(
                                psq, ones_bf, xsq[:, i, :],
                                start=(i == 0), stop=(i == 7))
                        mu = sb.tile([1, 512], F32, tag="mu", bufs=1)
                        rstd = sb.tile([1, 512], F32, tag="rstd", bufs=1)
                        msr = sb.tile([1, 512], F32, tag="msr", bufs=1)
                        nc.vector.tensor_scalar_mul(mu, psm, 1.0 / E)
                        nc.vector.tensor_scalar_mul(rstd, psq, 1.0 / E)
                        nc.vector.tensor_tensor(msr, mu, mu, MUL)
                        nc.vector.tensor_tensor(rstd, rstd, msr, SUB)
                        # rstd = 1/sqrt(var + eps)
                        nc.scalar.activation(rstd, rstd, SQRT, bias=eps_sb)
                        nc.vector.reciprocal(rstd, rstd)
                        nc.vector.tensor_tensor(msr, mu, rstd, MUL)
                        bc_rs = sb.tile([128, 512], F32, tag="bc_rs", bufs=2)
                        nc.gpsimd.partition_broadcast(bc_rs, rstd)
                        bc_ms = sb.tile([128, 512], F32, tag="bc_ms", bufs=2)
                        nc.gpsimd.partition_broadcast(bc_ms, msr)
                        # normalize in place: h = x*rstd - mu*rstd
                        nc.vector.tensor_tensor(
                            xTs[:, :, csl], xTs[:, :, csl],
                            bc_rs.unsqueeze(1).broadcast_to([128, ET, 512]),
                            MUL)
                        nc.vector.tensor_tensor(
                            xTs[:, :, csl], xTs[:, :, csl],
                            bc_ms.unsqueeze(1).broadcast_to([128, ET, 512]),
                            SUB)

                    def emit_qkv(c):
                        csl = ds(512 * c, 512)
                        for wi, (w_sb, o_sb) in enumerate(
                                ((w_q, QT), (w_k, KT))):
                            for m in range(4):
                                pq = ps.tile([128, 512], F32, tag="pq",
                                             bufs=2)
                                for i in range(ET):
                                    nc.tensor.matmul(
                                        pq, w_sb[:, i, ts(m, 128)],
                                        xTs[:, i, csl],
                                        start=(i == 0), stop=(i == 7))
                                if (wi * 4 + m) % 2 == 0:
                                    nc.vector.tensor_copy(
                                        o_sb[:, m, csl], pq)
                                else:
                                    nc.scalar.activation(
                                        o_sb[:, m, csl], pq, COPY)
                        for mt in range(4):
                            kt = 4 * c + mt
                            pv = ps.tile([128, 512], F32, tag="pv", bufs=2)
                            for i in range(ET):
                                nc.tensor.matmul(
                                    pv, xTs[:, i, ts(kt, 128)], w_v[:, i, :],
                                    start=(i == 0), stop=(i == 7))
                            vdst = Vp[:, kt, :, 0:64]
                            pvr = pv.rearrange("p (h d) -> p h d", h=HPC)
                            if mt % 2 == 0:
                                nc.vector.tensor_copy(vdst, pvr)
                            else:
                                nc.scalar.activation(vdst, pvr, COPY)

                    for c in range(4):
                        emit_stats(c)
                        if c > 0:
                            emit_qkv(c - 1)
                    emit_qkv(3)
                    # preload the exp table while QKV finishes
                    nc.scalar.activation(scratch1, scratch1, EXP)
                    if dbg:
                        nc.sync.dma_start(out=dbg_t["dXN"], in_=xTs)
                        nc.sync.dma_start(out=dbg_t["dQT"], in_=QT)
                        nc.sync.dma_start(out=dbg_t["dKT"], in_=KT)
                        nc.sync.dma_start(out=dbg_t["dVp"], in_=Vp)

                # ---- A2+A3: attention columns interleaved with proj/RS ----
                with tc.tile_pool(name="att", bufs=1) as sb, \
                     tc.tile_pool(name="att_ps", bufs=1,
                                  space="PSUM") as ps:
                    apws = sb.tile([128, 4, E], BF16, name="apws")
                    nc.sync.dma_start(
                        out=apws, in_=apw.rearrange("(k p) e -> p k e",
                                                    p=128))
                    xrs = sb.tile([128, ET, TH], F32, name="xrs")
                    nc.sync.dma_start(
                        out=xrs, in_=xrpT.rearrange("(i p) t -> p i t",
                                                    p=128))
                    def emit_norm(c, heads, rcl):
                        csl = ts(c, 512)
                        for h in heads:
                            hp, z = h // 2, h % 2
                            pp = slice(64 * z, 64 * z + 64)
                            lp = 32 * (h % 4)
                            # HW partition_broadcast reads the tile's
                            # partition 0 only -> bounce through a p0 row.
                            t0 = sb.tile([1, 512], F32, tag="t0", bufs=3)
                            nc.vector.tensor_copy(t0, rcl[lp:lp + 1, :])
                            rbc = sb.tile([128, 512], F32, tag="rbc", bufs=3)
                            nc.gpsimd.partition_broadcast(rbc, t0)
                            nc.vector.tensor_tensor(
                                AO[pp, hp, csl], AO[pp, hp, csl], rbc[pp, :],
                                MUL)

                    def emit_res_half(half, rsx):
                        hsl = ds(512 * half, 512)
                        rsb = sb.tile([128, ET, 512], BF16, tag="rsb",
                                      bufs=2)
                        nc.sync.dma_start(
                            out=rsb,
                            in_=rsx.rearrange("(j p) t -> p j t", p=128))
                        nc.vector.tensor_tensor(
                            xp[:, :, hsl], rsb, xrs[:, :, hsl], ADD)

                    for c in range(4):
                        csl = ts(c, 512)
                        njs = 4 * (c + 1)
                        for h in range(HPC):
                            hp, z = h // 2, h % 2
                            pp = slice(64 * z, 64 * z + 64)
                            psO = ps.tile([65, 512], F32, tag="psO",
                                          bufs=2, name="psO")
                            for g in range(njs // 2):
                                pS2 = ps.tile([128, 2, 512], F32, tag="pS",
                                              bufs=3, name="pS2")
                                for kk in range(2):
                                    nc.tensor.matmul(
                                        pS2[:, kk, :],
                                        KT[pp, hp, ts(2 * g + kk, 128)],
                                        QT[pp, hp, csl],
                                        start=True, stop=True)
                                PT2 = sb.tile([128, 2, 512], BF16, tag="PT",
                                              bufs=3, name="PT2")
                                nc.scalar.activation(
                                    PT2, pS2, EXP, scale=float(HS) ** -0.5)
                                if g >= 2 * c:
                                    md = 2 * (g - 2 * c)
                                    nc.vector.tensor_tensor(
                                        PT2, PT2, masks[:, md:md + 2, :],
                                        MUL)
                                for kk in range(2):
                                    j = 2 * g + kk
                                    nc.tensor.matmul(
                                        psO, Vp[:, j, h, :], PT2[:, kk, :],
                                        start=(j == 0), stop=(j == njs - 1))
                            # stage raw output + denominator; normalize later
                            nc.vector.tensor_copy(
                                AO[pp, hp, csl], psO[0:64, :])
                            lst = LstA if h < 4 else LstB
                            lp = 32 * (h % 4)
                            nc.vector.tensor_copy(
                                lst[lp:lp + 1, c, :], psO[64:65, :])
                            if h == 3:
                                # heads 0-3 normalize overlaps heads 4-7
                                rclA = sb.tile([97, 512], F32, tag="rclA",
                                               bufs=2)
                                nc.vector.reciprocal(rclA, LstA[:, c, :])
                                emit_norm(c, range(0, 4), rclA)
                            if h == 7:
                                rclB = sb.tile([97, 512], F32, tag="rclB",
                                               bufs=2)
                                nc.vector.reciprocal(rclB, LstB[:, c, :])
                                emit_norm(c, range(4, 8), rclB)
                        # projection for this column -> bf16 partials
                        dst = partA if c % 2 == 0 else partB
                        slot = c // 2
                        for em in range(ET):
                            pP = ps.tile([128, 512], F32, tag="pS", bufs=3,
                                         name="pP")
                            for kh in range(4):
                                nc.tensor.matmul(
                                    pP, apws[:, kh, ts(em, 128)],
                                    AO[:, kh, csl],
                                    start=(kh == 0), stop=(kh == 3))
                            po = sb.tile([128, 512], BF16, tag="po", bufs=4)
                            nc.vector.tensor_copy(po, pP)
                            nc.sync.dma_start(
                                out=dst[slot, ts(em, 128), :], in_=po)
                        if c == 2:
                            if single:
                                nc.sync.dma_start(out=rsA[:],
                                                  in_=partA[0, :, :])
                            else:
                                nc.gpsimd.collective_compute(
                                    "ReduceScatter", ADD,
                                    replica_groups=groups,
                                    ins=[partA[:]], outs=[rsA[:]])
                            # half-A residual overlaps column 3
                            emit_res_half(0, rsA)
                        if c == 3:
                            if single:
                                nc.sync.dma_start(out=rsB[:],
                                                  in_=partB[0, :, :])
                            else:
                                nc.gpsimd.collective_compute(
                                    "ReduceScatter", ADD,
                                    replica_groups=groups,
                                    ins=[partB[:]], outs=[rsB[:]])
                            emit_res_half(1, rsB)
                    if dbg:
                        nc.sync.dma_start(out=dbg_t["dAO"], in_=AO)
                        nc.sync.dma_start(out=dbg_t["dLA"], in_=LstA)
                        nc.sync.dma_start(out=dbg_t["dLB"], in_=LstB)
                        nc.sync.dma_start(out=dbg_t["drsA"], in_=rsA[:])
                        nc.sync.dma_start(out=dbg_t["drsB"], in_=rsB[:])

            # ======== phase B: LN2 + FFN (e-major), token-halved ========
            with tc.tile_pool(name="persB", bufs=1) as pB:
                nc.sync.dma_start(out=fb1_sb, in_=fb1)
                nc.sync.dma_start(out=fb2_sb, in_=fb2)
                if dbg:
                    nc.sync.dma_start(out=dbg_t["dxp"], in_=xp)
                h2T = pB.tile([128, ET, TH], BF16, name="h2T")
                ffh = pB.tile([128, FF // 128, TH], BF16, name="ffh")
                with tc.tile_pool(name="ffw", bufs=1) as sbw, \
                     tc.tile_pool(name="ff_ps", bufs=1, space="PSUM") as ps:
                    # LN2 + ff1 per token half
                    for half in range(2):
                        hsl = ds(512 * half, 512)
                        xpb = sbw.tile([128, ET, 512], BF16, tag="xpb",
                                       bufs=1)
                        nc.scalar.activation(xpb, xp[:, :, hsl], COPY)
                        xqb = sbw.tile([128, ET, 512], BF16, tag="xqb",
                                       bufs=1)
                        nc.vector.tensor_tensor(xqb, xpb, xpb, MUL)
                        psm = ps.tile([1, 512], F32, tag="psm2", bufs=1)
                        psq = ps.tile([1, 512], F32, tag="psq2", bufs=1)
                        for i in range(ET):
                            nc.tensor.matmul(
                                psm, ones_bf, xpb[:, i, :],
                                start=(i == 0), stop=(i == 7))
                        for i in range(ET):
                            nc.tensor.matmul(
                                psq, ones_bf, xqb[:, i, :],
                                start=(i == 0), stop=(i == 7))
                        mu = sbw.tile([1, 512], F32, tag="mu2", bufs=1)
                        rstd = sbw.tile([1, 512], F32, tag="rstd2", bufs=1)
                        msr = sbw.tile([1, 512], F32, tag="msr2", bufs=1)
                        nc.vector.tensor_scalar_mul(mu, psm, 1.0 / E)
                        nc.vector.tensor_scalar_mul(rstd, psq, 1.0 / E)
                        nc.vector.tensor_tensor(msr, mu, mu, MUL)
                        nc.vector.tensor_tensor(rstd, rstd, msr, SUB)
                        nc.scalar.activation(rstd, rstd, SQRT, bias=eps_sb)
                        nc.vector.reciprocal(rstd, rstd)
                        nc.vector.tensor_tensor(msr, mu, rstd, MUL)
                        bc_rs = sbw.tile([128, 512], F32, tag="bc_rs2",
                                         bufs=1)
                        nc.gpsimd.partition_broadcast(bc_rs, rstd)
                        bc_ms = sbw.tile([128, 512], F32, tag="bc_ms2",
                                         bufs=1)
                        nc.gpsimd.partition_broadcast(bc_ms, msr)
                        nc.vector.tensor_tensor(
                            h2T[:, :, hsl], xpb,
                            bc_rs.unsqueeze(1).broadcast_to([128, ET, 512]),
                            MUL)
                        nc.vector.tensor_tensor(
                            h2T[:, :, hsl], h2T[:, :, hsl],
                            bc_ms.unsqueeze(1).broadcast_to([128, ET, 512]),
                            SUB)
                        # ---- ff1 for this token half ----
                        for m in range(FF // 128):
                            if m % 4 == 0:
                                w1e = sbw.tile([128, ET, 512], BF16,
                                               tag="w1e", bufs=2)
                                nc.sync.dma_start(
                                    out=w1e,
                                    in_=fw1.rearrange(
                                        "(i p) f -> p i f",
                                        p=128)[:, :, ds(512 * (m // 4), 512)])
                            pF = ps.tile([128, 512], F32, tag="pF", bufs=3)
                            for i in range(ET):
                                nc.tensor.matmul(
                                    pF, w1e[:, i, ts(m % 4, 128)],
                                    h2T[:, i, hsl],
                                    start=(i == 0), stop=(i == 7))
                            nc.scalar.activation(
                                ffh[:, m, hsl], pF, RELU,
                                bias=fb1_sb[:, m:m + 1])
                    if dbg:
                        nc.sync.dma_start(out=dbg_t["dh2"], in_=h2T)
                    # ---- ff2 + residual + out, E-quarters x token halves ----
                    for eq in range(4):
                        w2q = sbw.tile([128, FF // 128, 256], BF16,
                                       tag="w2q", bufs=2)
                        nc.sync.dma_start(
                            out=w2q,
                            in_=fw2.rearrange("(k p) e -> p k e",
                                              p=128)[:, :,
                                                     ds(256 * eq, 256)])
                        for half in range(2):
                            hsl = ds(512 * half, 512)
                            for m2 in range(2):
                                m = 2 * eq + m2
                                pG = ps.tile([128, 512], F32, tag="pG",
                                             bufs=2)
                                for k in range(FF // 128):
                                    nc.tensor.matmul(
                                        pG, w2q[:, k, ts(m2, 128)],
                                        ffh[:, k, hsl],
                                        start=(k == 0), stop=(k == 31))
                                fin = sbw.tile([128, 512], F32, tag="fin",
                                               bufs=3)
                                nc.vector.tensor_tensor(
                                    fin, pG, xp[:, m, hsl], ADD)
                                nc.scalar.activation(
                                    fin, fin,
                                    mybir.ActivationFunctionType.Identity,
                                    bias=fb2_sb[:, m:m + 1])
                                nc.sync.dma_start(
                                    out=out.rearrange(
                                        "(i p) t -> p i t", p=128)[:, m, hsl],
                                    in_=fin)

    with tile.TileContext(nc) as tc:
        _emit(tc)

    nc.compile()
    return nc


_CACHED = {}


def _prepare_inputs(x, qkv_w, attn_proj_w, attn_proj_b, ln1_g, ln1_b,
                    ln2_g, ln2_b, ff_w1, ff_b1, ff_w2, ff_b2):
    """Fold LN affine params into the weights, shard, and cast to bf16."""
    x = np.asarray(x, np.float32)
    qkv_w = np.asarray(qkv_w, np.float32) * np.asarray(ln1_g, np.float32)[:, None]
    qkv_b = np.asarray(ln1_b, np.float32) @ qkv_w  # [3*H*HS]
    assert np.abs(qkv_b).max() == 0.0, "nonzero ln1_b not supported"
    ff_w1f = np.asarray(ff_w1, np.float32) * np.asarray(ln2_g, np.float32)[:, None]
    ff_b1f = np.asarray(ff_b1, np.float32) + np.asarray(ln2_b, np.float32) @ ff_w1f
    apb = np.asarray(attn_proj_b, np.float32)

    fw1_bf = ff_w1f.astype(NPBF16)
    fw2_bf = np.asarray(ff_w2, np.float32).astype(NPBF16)
    fb1_t = np.ascontiguousarray(ff_b1f.reshape(FF // 128, 128).T)
    fb2_t = np.ascontiguousarray(
        np.asarray(ff_b2, np.float32).reshape(ET, 128).T)
    apw_bf = np.asarray(attn_proj_w, np.float32).astype(NPBF16)

    in_maps = []
    for c in range(NCORES):
        b, hh = c // 2, c % 2
        hsl = slice(512 * hh, 512 * hh + 512)
        tsl = slice(TH * hh, TH * hh + TH)
        in_maps.append({
            "xT": np.ascontiguousarray(x[b].T).astype(NPBF16),
            "xrpT": np.ascontiguousarray((x[b, tsl] + apb[None, :]).T),
            "qw": np.ascontiguousarray(qkv_w[:, hsl]).astype(NPBF16),
            "kw": np.ascontiguousarray(qkv_w[:, H * HS:][:, hsl]).astype(NPBF16),
            "vw": np.ascontiguousarray(qkv_w[:, 2 * H * HS:][:, hsl]).astype(NPBF16),
            "apw": np.ascontiguousarray(apw_bf[hsl, :]),
            "fw1": fw1_bf,
            "fb1": fb1_t,
            "fw2": fw2_bf,
            "fb2": fb2_t,
        })
    return in_maps


def kernel(**inputs):
    if "nc" not in _CACHED:
        _CACHED["nc"] = build_program()
    nc = _CACHED["nc"]
    in_maps = _prepare_inputs(**inputs)
    res = run_bass_kernel_spmd(nc, in_maps, list(range(NCORES)))
    full = np.empty((B, T, E), np.float32)
    for c in range(NCORES):
        b, hh = c // 2, c % 2
        full[b, TH * hh:TH * hh + TH] = res.results[c]["out"].T
    return full


# revision 23
# speedup vs baseline: 1.8010x; 1.2203x over previous
"""Fused pre-LN transformer block (causal MHA + FFN) on 8 TRN2 NeuronCores.

Sharding: core c handles batch b = c//2 and head-half hh = c%2 (8 of 16 heads).
Attention runs fully local per (batch, head-half); the attention projection
produces a partial sum that is ReduceScattered (over token dim) within each
core pair, so FFN runs token-sharded (1024 tokens/core). Output per core is
its token slice, stored E-major [E, TH]; the host transposes when gathering.

v4: per-engine streams are executed in emission order, so the program is
emitted as one interleaved schedule:
- LN1+QKV software-pipelined over 4 token chunks.
- Attention emitted column-outer (q-chunk) x head with exp groups of 3
  k-blocks; the psO (attn-V) matmuls for group g are emitted after the
  scores of group g+1, so the PE never sits behind the scalar engine's exp.
- Softmax denominators staged to 8 partitions; one batched reciprocal per
  4 heads; normalization applied lazily in-place on the raw attn output.
- The attention projection for column c is emitted in 2-E-tile slices
  between the heads of column c+1 (fills the column-boundary bubble);
  the two bf16 pair ReduceScatters are issued mid-column-3 / right after.
- LN2 for token-half A is emitted inside the attention region (overlaps
  proj(3)/RS_B); LN2 for half B is woven into the middle of ff1(A).
- FFN: fw1 streamed in 1MB eighths per half, fw2 in quarters; relu and
  biases fused on the scalar engine.
"""

import numpy as np
import ml_dtypes

import concourse.bass as bass
import concourse.mybir as mybir
import concourse.tile as tile
from concourse import bacc
from concourse.bass import ts, ds
from concourse.bass_utils import run_bass_kernel_spmd

BF16 = mybir.dt.bfloat16
F32 = mybir.dt.float32
NPBF16 = ml_dtypes.bfloat16

B, T, E = 4, 2048, 1024
H, HS = 16, 64
FF = 4 * E
EPS = 1e-5
NCORES = 8
HPC = 8            # heads per core
HD = HPC * HS      # 512 head dims per core
TH = T // 2        # 1024 tokens per core for FFN
KT_N = T // 128    # 16 k-tiles
ET = E // 128      # 8 e-tiles
ADD = mybir.AluOpType.add
SUB = mybir.AluOpType.subtract
MUL = mybir.AluOpType.mult
EXP = mybir.ActivationFunctionType.Exp
RELU = mybir.ActivationFunctionType.Relu
SQRT = mybir.ActivationFunctionType.Sqrt
COPY = mybir.ActivationFunctionType.Copy
IDENT = mybir.ActivationFunctionType.Identity


def build_program(single=False, dbg=False):
    nc = bacc.Bacc("TRN2", target_bir_lowering=False, debug=False,
                   num_devices=1 if single else NCORES)
    dbg_t = {}
    if dbg:
        for name, shape, dt in (
                ("dQT", [128, 4, T], BF16), ("dKT", [128, 4, T], BF16),
                ("dVp", [128, KT_N, HPC, 65], BF16),
                ("dAO", [128, 4, T], BF16), ("dLA", [97, 4, 512], F32),
                ("dLB", [97, 4, 512], F32), ("drsA", [E, 512], BF16),
                ("drsB", [E, 512], BF16), ("dxp", [128, ET, TH], F32)):
            dbg_t[name] = nc.dram_tensor(name, shape, dt,
                                         kind="ExternalOutput").ap()

    # ---- I/O ----
    xT = nc.dram_tensor("xT", [E, T], BF16, kind="ExternalInput").ap()
    xrpT = nc.dram_tensor("xrpT", [E, TH], BF16, kind="ExternalInput").ap()
    qw = nc.dram_tensor("qw", [E, HD], BF16, kind="ExternalInput").ap()
    kw = nc.dram_tensor("kw", [E, HD], BF16, kind="ExternalInput").ap()
    vw = nc.dram_tensor("vw", [E, HD], BF16, kind="ExternalInput").ap()
    apw = nc.dram_tensor("apw", [HD, E], BF16, kind="ExternalInput").ap()
    fw1 = nc.dram_tensor("fw1", [E, FF], BF16, kind="ExternalInput").ap()
    fb1 = nc.dram_tensor("fb1", [128, FF // 128], F32,
                         kind="ExternalInput").ap()
    fw2 = nc.dram_tensor("fw2", [FF, E], BF16, kind="ExternalInput").ap()
    fb2 = nc.dram_tensor("fb2", [128, ET], F32, kind="ExternalInput").ap()
    out = nc.dram_tensor("out", [E, TH], F32, kind="ExternalOutput").ap()

    # internal DRAM for the two chunked pair-wise reduce-scatters (bf16).
    # partA holds global token quarters {0, 2}, partB {1, 3}; RS over the
    # core pair scatters dim0, so the even core gets quarters 0,1 and the
    # odd core quarters 2,3 == its own token half.
    partA = nc.dram_tensor("partA", [2, E, 512], BF16).ap()
    partB = nc.dram_tensor("partB", [2, E, 512], BF16).ap()
    rsA = nc.dram_tensor("rsA", [E, 512], BF16).ap()
    rsB = nc.dram_tensor("rsB", [E, 512], BF16).ap()
    groups = [[0, 1], [2, 3], [4, 5], [6, 7]]

    def _emit(tc):
        with tc.tile_pool(name="const", bufs=1) as constp:
            ones_bf = constp.tile([128, 1], BF16)
            nc.gpsimd.memset(ones_bf, 1.0)
            eps_sb = constp.tile([1, 1], F32)
            nc.gpsimd.memset(eps_sb, EPS)
            scratch1 = constp.tile([1, 64], F32)
            nc.gpsimd.memset(scratch1, 1.0)
            fb1_sb = constp.tile([128, FF // 128], F32)
            fb2_sb = constp.tile([128, ET], F32)

            # x' (post-attention residual input), lives to the end.
            xp = constp.tile([128, ET, TH], F32, name="xp")
            # LN2 output for token half A (computed in the attn region)
            h2A = constp.tile([128, ET, 512], BF16, name="h2A")

            def emit_ln2(half, pool, psp, pstag, dst):
                hsl = ds(512 * half, 512)
                xpb = pool.tile([128, ET, 512], BF16, tag=f"xpb{half}",
                                bufs=1)
                nc.scalar.activation(xpb, xp[:, :, hsl], COPY)
                xqb = pool.tile([128, ET, 512], BF16, tag=f"xqb{half}",
                                bufs=1)
                nc.vector.tensor_tensor(xqb, xpb, xpb, MUL)
                psm = psp.tile([1, 512], F32, tag=pstag, bufs=2)
                psq = psp.tile([1, 512], F32, tag=pstag, bufs=2)
                for i in range(ET):
                    nc.tensor.matmul(psm, ones_bf, xpb[:, i, :],
                                     start=(i == 0), stop=(i == 7))
                for i in range(ET):
                    nc.tensor.matmul(psq, ones_bf, xqb[:, i, :],
                                     start=(i == 0), stop=(i == 7))
                mu = pool.tile([1, 512], F32, tag=f"mu_{half}", bufs=1)
                rstd = pool.tile([1, 512], F32, tag=f"rstd_{half}", bufs=1)
                msr = pool.tile([1, 512], F32, tag=f"msr_{half}", bufs=1)
                nc.vector.tensor_scalar_mul(mu, psm, 1.0 / E)
                nc.vector.tensor_scalar_mul(rstd, psq, 1.0 / E)
                nc.vector.tensor_tensor(msr, mu, mu, MUL)
                nc.vector.tensor_tensor(rstd, rstd, msr, SUB)
                nc.scalar.activation(rstd, rstd, SQRT, bias=eps_sb)
                nc.vector.reciprocal(rstd, rstd)
                nc.vector.tensor_tensor(msr, mu, rstd, MUL)
                bc_rs = pool.tile([128, 512], F32, tag=f"bcr_{half}",
                                  bufs=1)
                nc.gpsimd.partition_broadcast(bc_rs, rstd)
                bc_ms = pool.tile([128, 512], F32, tag=f"bcm_{half}",
                                  bufs=1)
                nc.gpsimd.partition_broadcast(bc_ms, msr)
                nc.vector.tensor_tensor(
                    dst, xpb,
                    bc_rs.unsqueeze(1).broadcast_to([128, ET, 512]), MUL)
                nc.vector.tensor_tensor(
                    dst, dst,
                    bc_ms.unsqueeze(1).broadcast_to([128, ET, 512]), SUB)

            # ======== phase A: attention ========
            with tc.tile_pool(name="persA", bufs=1) as pA:
                QT = pA.tile([128, 4, T], BF16, name="QT")
                KT = pA.tile([128, 4, T], BF16, name="KT")
                AO = pA.tile([128, 4, T], BF16, name="AO")
                Vp = pA.tile([128, KT_N, HPC, 65], BF16, name="Vp")
                nc.vector.memset(Vp[:, :, :, 64:65], 1.0)
                # softmax denominators: 4 heads per tile on partitions
                # {0,32,64,96} (DVE partition-start constraint), so the
                # per-column reciprocal batches 4 heads per op.
                LstA = pA.tile([97, 4, 512], F32, name="LstA")
                LstB = pA.tile([97, 4, 512], F32, name="LstB")
                nc.vector.memset(LstA, 1.0)
                nc.vector.memset(LstB, 1.0)
                # causal masks for the 4 diagonal k-offsets:
                # masks[p, kk, qq] = 1 if qq >= 128*kk + p else 0
                masks = pA.tile([128, 4, 512], BF16, name="masks")
                nc.gpsimd.memset(masks, 1.0)
                nc.gpsimd.affine_select(
                    out=masks, in_=masks, compare_op=mybir.AluOpType.is_ge,
                    fill=0.0, base=0, pattern=[[-128, 4], [1, 512]],
                    channel_multiplier=-1)

                # ---- A1: LN1 + QKV, software-pipelined over 4 chunks ----
                with tc.tile_pool(name="ln1", bufs=1) as sb, \
                     tc.tile_pool(name="ln1_ps", bufs=1, space="PSUM") as ps:
                    xTs = sb.tile([128, ET, T], BF16, name="xTs")
                    w_q = sb.tile([128, ET, HD], BF16, name="w_q")
                    w_k = sb.tile([128, ET, HD], BF16, name="w_k")
                    w_v = sb.tile([128, ET, HD], BF16, name="w_v")
                    nc.sync.dma_start(
                        out=xTs[:, :, ds(0, 512)],
                        in_=xT.rearrange("(i p) t -> p i t",
                                         p=128)[:, :, ds(0, 512)])
                    nc.sync.dma_start(
                        out=w_q, in_=qw.rearrange("(i p) f -> p i f", p=128))
                    nc.sync.dma_start(
                        out=w_k, in_=kw.rearrange("(i p) f -> p i f", p=128))
                    nc.sync.dma_start(
                        out=w_v, in_=vw.rearrange("(i p) f -> p i f", p=128))

                    def emit_stats(c):
                        csl = ds(512 * c, 512)
                        if c > 0:
                            nc.sync.dma_start(
                                out=xTs[:, :, csl],
                                in_=xT.rearrange("(i p) t -> p i t",
                                                 p=128)[:, :, csl])
                        xsq = sb.tile([128, ET, 512], BF16, tag="xsq",
                                      bufs=1)
                        nc.vector.tensor_tensor(
                            xsq, xTs[:, :, csl], xTs[:, :, csl], MUL)
                        psm = ps.tile([1, 512], F32, tag="psm", bufs=2)
                        psq = ps.tile([1, 512], F32, tag="psq", bufs=2)
                        for i in range(ET):
                            nc.tensor.matmul(
                                psm, ones_bf, xTs[:, i, csl],
                                start=(i == 0), stop=(i == 7))
                        for i in range(ET):
                            nc.tensor.matmul(
                                psq, ones_bf, xsq[:, i, :],
                                start=(i == 0), stop=(i == 7))
                        mu = sb.tile([1, 512], F32, tag="mu", bufs=1)
                        rstd = sb.tile([1, 512], F32, tag="rstd", bufs=1)
                        msr = sb.tile([1, 512], F32, tag="msr", bufs=1)
                        nc.vector.tensor_scalar_mul(mu, psm, 1.0 / E)
                        nc.vector.tensor_scalar_mul(rstd, psq, 1.0 / E)
                        nc.vector.tensor_tensor(msr, mu, mu, MUL)
                        nc.vector.tensor_tensor(rstd, rstd, msr, SUB)
                        # rstd = 1/sqrt(var + eps)
                        nc.scalar.activation(rstd, rstd, SQRT, bias=eps_sb)
                        nc.vector.reciprocal(rstd, rstd)
                        nc.vector.tensor_tensor(msr, mu, rstd, MUL)
                        bc_rs = sb.tile([128, 512], F32, tag="bc_rs", bufs=2)
                        nc.gpsimd.partition_broadcast(bc_rs, rstd)
                        bc_ms = sb.tile([128, 512], F32, tag="bc_ms", bufs=2)
                        nc.gpsimd.partition_broadcast(bc_ms, msr)
                        # normalize in place: h = x*rstd - mu*rstd
                        nc.vector.tensor_tensor(
                            xTs[:, :, csl], xTs[:, :, csl],
                            bc_rs.unsqueeze(1).broadcast_to([128, ET, 512]),
                            MUL)
                        nc.vector.tensor_tensor(
                            xTs[:, :, csl], xTs[:, :, csl],
                            bc_ms.unsqueeze(1).broadcast_to([128, ET, 512]),
                            SUB)

                    def emit_qkv(c):
                        csl = ds(512 * c, 512)
                        for wi, (w_sb, o_sb) in enumerate(
                                ((w_q, QT), (w_k, KT))):
                            for m in range(4):
                                pq = ps.tile([128, 512], F32, tag="pq",
                                             bufs=2)
                                for i in range(ET):
                                    nc.tensor.matmul(
                                        pq, w_sb[:, i, ts(m, 128)],
                                        xTs[:, i, csl],
                                        start=(i == 0), stop=(i == 7))
                                if (wi * 4 + m) % 2 == 0:
                                    nc.vector.tensor_copy(
                                        o_sb[:, m, csl], pq)
                                else:
                                    nc.scalar.activation(
                                        o_sb[:, m, csl], pq, COPY)
                        for mt in range(4):
                            kt = 4 * c + mt
                            pv = ps.tile([128, 512], F32, tag="pv", bufs=2)
                            for i in range(ET):
                                nc.tensor.matmul(
                                    pv, xTs[:, i, ts(kt, 128)], w_v[:, i, :],
                                    start=(i == 0), stop=(i == 7))
                            vdst = Vp[:, kt, :, 0:64]
                            pvr = pv.rearrange("p (h d) -> p h d", h=HPC)
                            if mt % 2 == 0:
                                nc.vector.tensor_copy(vdst, pvr)
                            else:
                                nc.scalar.activation(vdst, pvr, COPY)

                    for c in range(4):
                        emit_stats(c)
                        if c > 0:
                            emit_qkv(c - 1)
                    emit_qkv(3)
                    # preload the exp table while QKV finishes
                    nc.scalar.activation(scratch1, scratch1, EXP)

                # ---- A2+A3: attention/proj/RS/LN2-A interleaved ----
                with tc.tile_pool(name="att", bufs=1) as sb, \
                     tc.tile_pool(name="att_ps", bufs=1,
                                  space="PSUM") as ps:
                    apws = sb.tile([128, 4, E], BF16, name="apws")
                    nc.sync.dma_start(
                        out=apws, in_=apw.rearrange("(k p) e -> p k e",
                                                    p=128))
                    xrs = sb.tile([128, ET, TH], BF16, name="xrs")
                    nc.sync.dma_start(
                        out=xrs, in_=xrpT.rearrange("(i p) t -> p i t",
                                                    p=128))
                    G = 3

                    def emit_norm(c, heads, rcl):
                        csl = ts(c, 512)
                        for h in heads:
                            hp, z = h // 2, h % 2
                            pp = slice(64 * z, 64 * z + 64)
                            lp = 32 * (h % 4)
                            # HW partition_broadcast reads the tile's
                            # partition 0 only -> bounce through a p0 row.
                            t0 = sb.tile([1, 512], F32, tag="t0", bufs=2)
                            nc.vector.tensor_copy(t0, rcl[lp:lp + 1, :])
                            rbc = sb.tile([128, 512], F32, tag="rbc",
                                          bufs=2)
                            nc.gpsimd.partition_broadcast(rbc, t0)
                            nc.vector.tensor_tensor(
                                AO[pp, hp, csl], AO[pp, hp, csl],
                                rbc[pp, :], MUL)

                    def emit_head(c, h):
                        csl = ts(c, 512)
                        njs = 4 * (c + 1)
                        hp, z = h // 2, h % 2
                        pp = slice(64 * z, 64 * z + 64)
                        psO = ps.tile([65, 512], F32, tag="psO", bufs=2,
                                      name="psO")

                        def psO_mm(js, PT):
                            for idx, j in enumerate(js):
                                nc.tensor.matmul(
                                    psO, Vp[:, j, h, :], PT[:, idx, :],
                                    start=(j == 0), stop=(j == njs - 1))

                        pend = None
                        for g0 in range(0, njs, G):
                            js = list(range(g0, min(g0 + G, njs)))
                            n = len(js)
                            pS = ps.tile([128, G, 512], F32, tag="pS",
                                         bufs=2, name="pS")
                            for idx, j in enumerate(js):
                                nc.tensor.matmul(
                                    pS[:, idx, :], KT[pp, hp, ts(j, 128)],
                                    QT[pp, hp, csl], start=True, stop=True)
                            PT = sb.tile([128, G, 512], BF16, tag="PT",
                                         bufs=2, name="PT")
                            nc.scalar.activation(
                                PT[:, 0:n, :], pS[:, 0:n, :], EXP,
                                scale=float(HS) ** -0.5)
                            d0 = 4 * c
                            if js[-1] >= d0:
                                lo = max(js[0], d0)
                                a = lo - js[0]
                                nc.vector.tensor_tensor(
                                    PT[:, a:n, :], PT[:, a:n, :],
                                    masks[:, lo - d0:js[-1] - d0 + 1, :],
                                    MUL)
                            if pend is not None:
                                psO_mm(*pend)
                            pend = (js, PT)
                        psO_mm(*pend)
                        # stage raw output + denominator; normalize later
                        nc.vector.tensor_copy(AO[pp, hp, csl], psO[0:64, :])
                        lst = LstA if h < 4 else LstB
                        lp = 32 * (h % 4)
                        nc.vector.tensor_copy(lst[lp:lp + 1, c, :],
                                              psO[64:65, :])
                        if h == 3:
                            # heads 0-3 normalize overlaps heads 4-7
                            rclA = sb.tile([97, 512], F32, tag="rclA",
                                           bufs=1)
                            nc.vector.reciprocal(rclA, LstA[:, c, :])
                            emit_norm(c, range(0, 4), rclA)
                        if h == 7:
                            rclB = sb.tile([97, 512], F32, tag="rclB",
                                           bufs=1)
                            nc.vector.reciprocal(rclB, LstB[:, c, :])
                            emit_norm(c, range(4, 8), rclB)

                    def emit_proj(c, ems):
                        csl = ts(c, 512)
                        dst = partA if c % 2 == 0 else partB
                        slot = c // 2
                        for em in ems:
                            pP = ps.tile([128, 512], F32, tag="pS", bufs=2,
                                         name="pP")
                            for kh in range(4):
                                nc.tensor.matmul(
                                    pP, apws[:, kh, ts(em, 128)],
                                    AO[:, kh, csl],
                                    start=(kh == 0), stop=(kh == 3))
                            po = sb.tile([128, 512], BF16, tag="po", bufs=3)
                            nc.vector.tensor_copy(po, pP)
                            nc.sync.dma_start(
                                out=dst[slot, ts(em, 128), :], in_=po)

                    def emit_rs(part, rsx):
                        if single:
                            nc.sync.dma_start(out=rsx[:], in_=part[0, :, :])
                        else:
                            nc.gpsimd.collective_compute(
                                "ReduceScatter", ADD, replica_groups=groups,
                                ins=[part[:]], outs=[rsx[:]])

                    def emit_res_half(half, rsx):
                        hsl = ds(512 * half, 512)
                        rsb = sb.tile([128, ET, 512], BF16, tag="rsb",
                                      bufs=1)
                        nc.sync.dma_start(
                            out=rsb,
                            in_=rsx.rearrange("(j p) t -> p j t", p=128))
                        nc.vector.tensor_tensor(
                            xp[:, :, hsl], rsb, xrs[:, :, hsl], ADD)

                    for c in range(4):
                        for h in range(HPC):
                            emit_head(c, h)
                            # weave the previous column's projection in
                            # 2-E-tile slices between heads 1..4
                            if c >= 1 and 1 <= h <= 4:
                                emit_proj(c - 1, range(2 * (h - 1),
                                                       2 * h))
                                if c == 3 and h == 4:
                                    emit_rs(partA, rsA)
                    # column 3 epilogue: residual A, proj(3), RS_B, LN2-A
                    emit_res_half(0, rsA)
                    emit_proj(3, range(ET))
                    emit_rs(partB, rsB)
                    emit_ln2(0, sb, ps, "pS", h2A)
                    emit_res_half(1, rsB)
                    if dbg:
                        nc.sync.dma_start(out=dbg_t["dAO"], in_=AO)
                        nc.sync.dma_start(out=dbg_t["dLA"], in_=LstA)
                        nc.sync.dma_start(out=dbg_t["dLB"], in_=LstB)
                        nc.sync.dma_start(out=dbg_t["drsA"], in_=rsA[:])
                        nc.sync.dma_start(out=dbg_t["drsB"], in_=rsB[:])
                        nc.sync.dma_start(out=dbg_t["dQT"], in_=QT)
                        nc.sync.dma_start(out=dbg_t["dKT"], in_=KT)
                        nc.sync.dma_start(out=dbg_t["dVp"], in_=Vp)

            # ======== phase B: FFN (e-major), token-halved ========
            with tc.tile_pool(name="persB", bufs=1) as pB:
                nc.sync.dma_start(out=fb1_sb, in_=fb1)
                nc.sync.dma_start(out=fb2_sb, in_=fb2)
                if dbg:
                    nc.sync.dma_start(out=dbg_t["dxp"], in_=xp)
                h2B = pB.tile([128, ET, 512], BF16, name="h2B")
                ffh = pB.tile([128, FF // 128, TH], BF16, name="ffh")
                with tc.tile_pool(name="ffw", bufs=1) as sbw, \
                     tc.tile_pool(name="ff_ps", bufs=1, space="PSUM") as ps:
                    for half, h2 in ((0, h2A), (1, h2B)):
                        hsl = ds(512 * half, 512)
                        for m in range(FF // 128):
                            if half == 0 and m == 12:
                                # LN2 for half B hides inside ff1(A)
                                emit_ln2(1, sbw, ps, "pstat2", h2B)
                            if m % 4 == 0:
                                w1e = sbw.tile([128, ET, 512], BF16,
                                               tag="w1e", bufs=2)
                                nc.sync.dma_start(
                                    out=w1e,
                                    in_=fw1.rearrange(
                                        "(i p) f -> p i f",
                                        p=128)[:, :, ds(512 * (m // 4), 512)])
                            pF = ps.tile([128, 512], F32, tag="pF", bufs=3)
                            for i in range(ET):
                                nc.tensor.matmul(
                                    pF, w1e[:, i, ts(m % 4, 128)],
                                    h2[:, i, :],
                                    start=(i == 0), stop=(i == 7))
                            nc.scalar.activation(
                                ffh[:, m, hsl], pF, RELU,
                                bias=fb1_sb[:, m:m + 1])
                    # ---- ff2 + residual + out, E-quarters x token halves ----
                    for eq in range(4):
                        w2q = sbw.tile([128, FF // 128, 256], BF16,
                                       tag="w2q", bufs=2)
                        nc.sync.dma_start(
                            out=w2q,
                            in_=fw2.rearrange("(k p) e -> p k e",
                                              p=128)[:, :,
                                                     ds(256 * eq, 256)])
                        for half in range(2):
                            hsl = ds(512 * half, 512)
                            for m2 in range(2):
                                m = 2 * eq + m2
                                pG = ps.tile([128, 512], F32, tag="pG",
                                             bufs=2)
                                for k in range(FF // 128):
                                    nc.tensor.matmul(
                                        pG, w2q[:, k, ts(m2, 128)],
                                        ffh[:, k, hsl],
                                        start=(k == 0), stop=(k == 31))
                                fin = sbw.tile([128, 512], F32, tag="fin",
                                               bufs=3)
                                nc.vector.tensor_tensor(
                                    fin, pG, xp[:, m, hsl], ADD)
                                nc.scalar.activation(
                                    fin, fin, IDENT,
                                    bias=fb2_sb[:, m:m + 1])
                                nc.sync.dma_start(
                                    out=out.rearrange(
                                        "(i p) t -> p i t", p=128)[:, m, hsl],
                                    in_=fin)

    with tile.TileContext(nc) as tc:
        _emit(tc)

    nc.compile()
    return nc


_CACHED = {}


def _prepare_inputs(x, qkv_w, attn_proj_w, attn_proj_b, ln1_g, ln1_b,
                    ln2_g, ln2_b, ff_w1, ff_b1, ff_w2, ff_b2):
    """Fold LN affine params into the weights, shard, and cast to bf16."""
    x = np.asarray(x, np.float32)
    qkv_w = np.asarray(qkv_w, np.float32) * np.asarray(ln1_g, np.float32)[:, None]
    qkv_b = np.asarray(ln1_b, np.float32) @ qkv_w  # [3*H*HS]
    assert np.abs(qkv_b).max() == 0.0, "nonzero ln1_b not supported"
    ff_w1f = np.asarray(ff_w1, np.float32) * np.asarray(ln2_g, np.float32)[:, None]
    ff_b1f = np.asarray(ff_b1, np.float32) + np.asarray(ln2_b, np.float32) @ ff_w1f
    apb = np.asarray(attn_proj_b, np.float32)

    fw1_bf = ff_w1f.astype(NPBF16)
    fw2_bf = np.asarray(ff_w2, np.float32).astype(NPBF16)
    fb1_t = np.ascontiguousarray(ff_b1f.reshape(FF // 128, 128).T)
    fb2_t = np.ascontiguousarray(
        np.asarray(ff_b2, np.float32).reshape(ET, 128).T)
    apw_bf = np.asarray(attn_proj_w, np.float32).astype(NPBF16)

    in_maps = []
    for c in range(NCORES):
        b, hh = c // 2, c % 2
        hsl = slice(512 * hh, 512 * hh + 512)
        tsl = slice(TH * hh, TH * hh + TH)
        in_maps.append({
            "xT": np.ascontiguousarray(x[b].T).astype(NPBF16),
            "xrpT": np.ascontiguousarray(
                (x[b, tsl] + apb[None, :]).T).astype(NPBF16),
            "qw": np.ascontiguousarray(qkv_w[:, hsl]).astype(NPBF16),
            "kw": np.ascontiguousarray(qkv_w[:, H * HS:][:, hsl]).astype(NPBF16),
            "vw": np.ascontiguousarray(qkv_w[:, 2 * H * HS:][:, hsl]).astype(NPBF16),
            "apw": np.ascontiguousarray(apw_bf[hsl, :]),
            "fw1": fw1_bf,
            "fb1": fb1_t,
            "fw2": fw2_bf,
            "fb2": fb2_t,
        })
    return in_maps


def kernel(**inputs):
    if "nc" not in _CACHED:
        _CACHED["nc"] = build_program()
    nc = _CACHED["nc"]
    in_maps = _prepare_inputs(**inputs)
    res = run_bass_kernel_spmd(nc, in_maps, list(range(NCORES)))
    full = np.empty((B, T, E), np.float32)
    for c in range(NCORES):
        b, hh = c // 2, c % 2
        full[b, TH * hh:TH * hh + TH] = res.results[c]["out"].T
    return full
